# revision 1
# baseline (speedup 1.0000x reference)
"""Trainium2 Bass kernel for nn_DGMMLoss (retrieval_knn).

Reference computation (see problem statement):
  1. x_ul = lam*x + (1-lam)*x[perm]; pseudo-label via mode of 11-NN labels
  2. concat; per-class means; gaussian-mixture loss term
  3. kNN regularizer: mode of 3-NN (self-excluded) labels, MSE
  loss = loss_gm + 0.01 * loss_knn

Device strategy (8 NeuronCores, data-parallel over query rows; two SPMD
launches, phase A = 11-NN pseudo-labels, phase B = 3-NN mode + gm rows):
  - Scores s[q,r] = q.r - ||r||^2/2 via bf16 matmuls (fp32 psum); the -bb/2
    term rides in the same accumulation as an augmented K=2 contraction of a
    ones column against a bf16 hi/lo split of -bb/2 (exact to ~2^-17 rel),
    so psum evacuation is a pure copy and runs on the ACT engine.
  - Per-row k-th largest via DVE max8 (+match_replace+max8 for k=11) gives a
    per-partition threshold; one tensor_scalar is_ge produces the bf16
    mask[q,r] per 128-query block (two halves for finer pipelining).
  - Per-class counts = maskT.T @ onehot(y) on the PE: mask tiles are
    transposed on the PE (bf16, via identity), batched 8 per PSUM bank,
    evacuated by single ACT copies; onehot(y) is built on device from packed
    labels. Blocks are software-pipelined: block b's counts are emitted after
    block b+1's threshold so PE work overlaps the DVE tail.
  - mode = first argmax of counts (= smallest class on ties, matching
    torch.mode), via reduce_max / is_lt / reduce_min on DVE.
  - GM branch: pi = exp(q.mu - aa/2)*exp(-||mu||^2/2)*(counts>0),
    row-normalized; per-row sum((pi - onehot)^2) on device.
Host does only O(N*D) glue: x_ul, norms, packing, per-class means, final
scalar assembly. bf16 scoring shifts the loss by ~9e-4 relative (verified
against an fp64 model; fp32 matmul on TRN2 is 4x slower than bf16).
"""

from contextlib import ExitStack

import numpy as np
import ml_dtypes

import time as _time

import concourse.bacc as bacc
import concourse.tile as tile
import concourse.mybir as mybir
from concourse.bass_utils import run_bass_kernel_spmd
from concourse.masks import make_identity

P = 128
NCORES = 8
CLASSES = 100
F32 = mybir.dt.float32
BF16 = mybir.dt.bfloat16
BF16_NP = ml_dtypes.bfloat16
ALU = mybir.AluOpType
AX = mybir.AxisListType


def build_program(R, Q, D, C, k, self_exclude, gm, n_cores=NCORES, _stages=3):
    """One phase of the pipeline as a Bass/Tile program (SPMD over cores).

    R: number of reference rows (shared across cores)
    Q: number of query rows handled by this core
    k: keep the k nearest (largest score) refs per query row
    self_exclude: subtract the query's own label from the counts (knn branch)
    gm: also compute the per-row gaussian-mixture loss term
    """
    DCH, RT, RCH, QB = D // P, R // P, R // 512, Q // P
    assert D % P == 0 and R % 512 == 0 and Q % P == 0 and k <= 16

    nc = bacc.Bacc(
        "TRN2", target_bir_lowering=False, debug=False, num_devices=n_cores
    )
    xT_ap = nc.dram_tensor("xT", [P, DCH * R], BF16, kind="ExternalInput").ap()
    qT_ap = nc.dram_tensor("qT", [P, DCH * Q], BF16, kind="ExternalInput").ap()
    # -||r||^2/2 split into bf16 hi+lo rows, folded into the score matmul as
    # an augmented K=2 contraction against a column of ones.
    bb_ap = nc.dram_tensor("bbhl", [2, R], BF16, kind="ExternalInput").ap()
    # reference labels packed [P, RT]: column i holds y[i*128 : (i+1)*128]
    yl_ap = nc.dram_tensor("ylab", [P, RT], F32, kind="ExternalInput").ap()
    io_ap = nc.dram_tensor("iotaf", [P, C], F32, kind="ExternalInput").ap()
    nqaux = (2 * QB) if gm else QB
    qaux_ap = (
        nc.dram_tensor("qaux", [P, nqaux], F32, kind="ExternalInput").ap()
        if (self_exclude or gm)
        else None
    )
    muT_ap = emu_ap = None
    if gm:
        muT_ap = nc.dram_tensor("muT", [P, DCH * C], BF16, kind="ExternalInput").ap()
        emu_ap = nc.dram_tensor("emu", [P, C], F32, kind="ExternalInput").ap()
    ym_ap = nc.dram_tensor("ymode", [QB, P, 1], F32, kind="ExternalOutput").ap()
    lg_ap = (
        nc.dram_tensor("lgm", [QB, P, 1], F32, kind="ExternalOutput").ap()
        if gm
        else None
    )

    with tile.TileContext(nc) as tc, ExitStack() as ctx:
        consts = ctx.enter_context(tc.tile_pool(name="consts", bufs=1))
        sbig = ctx.enter_context(tc.tile_pool(name="sbig", bufs=2))
        maskp = ctx.enter_context(tc.tile_pool(name="maskp", bufs=1))
        small = ctx.enter_context(tc.tile_pool(name="small", bufs=1))
        psS_p = ctx.enter_context(tc.tile_pool(name="psS", bufs=3, space="PSUM"))
        psT_p = ctx.enter_context(tc.tile_pool(name="psT", bufs=2, space="PSUM"))
        psC_p = ctx.enter_context(tc.tile_pool(name="psC", bufs=1, space="PSUM"))
        psG_p = (
            ctx.enter_context(tc.tile_pool(name="psG", bufs=1, space="PSUM"))
            if gm
            else None
        )

        identb = consts.tile([P, P], BF16, name="identb", tag="identb")
        make_identity(nc, identb)

        # Tiny "touch" ops absorb DMA-queue waits into dedicated copies so the
        # wide compute instructions (1-2 HW wait slots) only wait on engine
        # semaphores.
        tchV = consts.tile([1, 1], F32, name="tchV", tag="tchV")
        tchA = consts.tile([1, 1], F32, name="tchA", tag="tchA")

        def dve_touch(ap):
            nc.vector.tensor_copy(tchV[:], ap[0:1, 0:1])

        def act_touch(ap):
            nc.scalar.copy(tchA[:], ap[0:1, 0:1])

        # PE touch of the identity so later transposes don't carry its wait.
        psI = psT_p.tile([1, P], BF16, name="psI", tag="psMI", bufs=1)
        nc.tensor.transpose(psI[:], identb[:, 0:1], identb[:])

        # DMA constants in. One dma_start per tile (Tile deps are per-tile, and
        # matmuls only have ~2 wait slots); big constants are split into
        # separate ref-group tiles so compute can start after the first group.
        GROUP = min(R, 1024)
        NG = R // GROUP
        xTs = [[None] * NG for _ in range(DCH)]
        for g in range(NG):
            for d in range(DCH):
                t = consts.tile(
                    [P, GROUP], BF16, name=f"xTs{d}_{g}", tag=f"xTs{d}_{g}"
                )
                nc.sync.dma_start(
                    t[:], xT_ap[:, d * R + g * GROUP: d * R + (g + 1) * GROUP]
                )
                xTs[d][g] = t
        qTt = consts.tile([P, DCH * Q], BF16, name="qTt", tag="qTt")
        nc.sync.dma_start(qTt[:], qT_ap[:])
        ones2 = consts.tile([2, P], BF16, name="ones2", tag="ones2")
        nc.vector.memset(ones2[:], 1.0)
        bbts = []
        for g in range(NG):
            t = consts.tile([2, GROUP], BF16, name=f"bbt{g}", tag=f"bbt{g}")
            nc.sync.dma_start(t[:], bb_ap[:, g * GROUP:(g + 1) * GROUP])
            bbts.append(t)
        ylabt = consts.tile([P, RT], F32, name="ylabt", tag="ylabt")
        nc.sync.dma_start(ylabt[:], yl_ap[:])
        iot = consts.tile([P, C], F32, name="iot", tag="iot")
        nc.sync.dma_start(iot[:], io_ap[:])
        if qaux_ap is not None:
            qauxt = consts.tile([P, nqaux], F32, name="qauxt", tag="qauxt")
            nc.sync.dma_start(qauxt[:], qaux_ap[:])
        if gm:
            muTt = consts.tile([P, DCH * C], BF16, name="muTt", tag="muTt")
            nc.sync.dma_start(muTt[:], muT_ap[:])
            emut = consts.tile([P, C], F32, name="emut", tag="emut")
            nc.sync.dma_start(emut[:], emu_ap[:])
        dve_touch(iot)
        dve_touch(ylabt)
        # one-hot labels built on device: yoht[:, i*C:(i+1)*C] = (iota == y_r)
        yoht = consts.tile([P, RT * C], BF16, name="yoht", tag="yoht")
        for i in range(RT):
            nc.vector.tensor_scalar(
                out=yoht[:, i * C:(i + 1) * C], in0=iot[:],
                scalar1=ylabt[:, i:i + 1], scalar2=None, op0=ALU.is_equal,
            )
        if qaux_ap is not None:
            dve_touch(qauxt)
            act_touch(qauxt)
        if gm:
            dve_touch(emut)

        R2 = R // 2
        HT = RT // 2  # mask tiles per half

        def emit_counts(b, halves):
            """Counts + mode (+ gm) for query block b given its mask halves."""
            psc = psC_p.tile([P, C], F32, name="psC", tag="psC")
            GT = min(8, RT)  # transposes batched per PSUM bank / ACT copy
            for i0 in range(0, RT, GT):
                pst = psT_p.tile([P, GT * P], BF16, name="psT", tag="psT")
                for u in range(GT):
                    i = i0 + u
                    mh = halves[i // HT]
                    lo = (i % HT) * P
                    nc.tensor.transpose(
                        pst[:, u * P:(u + 1) * P], mh[:, lo:lo + P], identb[:]
                    )
                mTg = maskp.tile([P, GT * P], BF16, name="mTg", tag="mTg", bufs=3)
                nc.scalar.copy(mTg[:], pst[:])
                for u in range(GT):
                    i = i0 + u
                    nc.tensor.matmul(
                        psc[:],
                        mTg[:, u * P:(u + 1) * P],
                        yoht[:, i * C:(i + 1) * C],
                        start=(i == 0),
                        stop=(i == RT - 1),
                    )
            counts = small.tile([P, C], F32, name="counts", tag="counts")
            if self_exclude or gm:
                yh = small.tile([P, C], F32, name="yh", tag="yh")
                nc.vector.tensor_scalar(
                    out=yh[:],
                    in0=iot[:],
                    scalar1=qauxt[:, b:b + 1],
                    scalar2=None,
                    op0=ALU.is_equal,
                )
            if self_exclude:
                nc.vector.tensor_sub(counts[:], psc[:], yh[:])
            else:
                nc.vector.tensor_copy(counts[:], psc[:])
            # mode = first argmax of counts
            maxc = small.tile([P, 1], F32, name="maxc", tag="maxc")
            nc.vector.reduce_max(maxc[:], counts[:], axis=AX.X)
            lt01 = small.tile([P, C], F32, name="lt01", tag="lt01")
            nc.vector.tensor_scalar(
                out=lt01[:], in0=counts[:], scalar1=maxc[:], scalar2=None,
                op0=ALU.is_lt,
            )
            cand = small.tile([P, C], F32, name="cand", tag="cand")
            nc.vector.scalar_tensor_tensor(
                out=cand[:], in0=lt01[:], scalar=1e9, in1=iot[:],
                op0=ALU.mult, op1=ALU.add,
            )
            ym = small.tile([P, 1], F32, name="ym", tag="ym")
            nc.vector.tensor_reduce(ym[:], cand[:], axis=AX.X, op=ALU.min)
            nc.sync.dma_start(ym_ap[b], ym[:])
            # gaussian-mixture per-row loss
            if gm:
                psg = psG_p.tile([P, C], F32, name="psG", tag="psG")
                for d in range(DCH):
                    nc.tensor.matmul(
                        psg[:],
                        qTt[:, d * Q + b * P: d * Q + (b + 1) * P],
                        muTt[:, d * C:(d + 1) * C],
                        start=(d == 0),
                        stop=(d == DCH - 1),
                    )
                eg = small.tile([P, C], F32, name="eg", tag="eg")
                nc.scalar.activation(
                    eg[:], psg[:], mybir.ActivationFunctionType.Exp,
                    bias=qauxt[:, QB + b:QB + b + 1], scale=1.0,
                )
                piu = small.tile([P, C], F32, name="piu", tag="piu")
                nc.vector.tensor_mul(piu[:], eg[:], emut[:])
                srow = small.tile([P, 1], F32, name="srow", tag="srow")
                nc.vector.reduce_sum(srow[:], piu[:], axis=AX.X)
                nc.vector.tensor_scalar_add(srow[:], srow[:], 1e-15)
                rec = small.tile([P, 1], F32, name="rec", tag="rec")
                nc.vector.reciprocal(rec[:], srow[:])
                pin = small.tile([P, C], F32, name="pin", tag="pin")
                nc.vector.tensor_scalar(
                    out=pin[:], in0=piu[:], scalar1=rec[:], scalar2=None,
                    op0=ALU.mult,
                )
                diff = small.tile([P, C], F32, name="diff", tag="diff")
                nc.vector.tensor_sub(diff[:], pin[:], yh[:])
                sq = small.tile([P, C], F32, name="sq", tag="sq")
                nc.vector.tensor_mul(sq[:], diff[:], diff[:])
                lg = small.tile([P, 1], F32, name="lg", tag="lg")
                nc.vector.reduce_sum(lg[:], sq[:], axis=AX.X)
                nc.sync.dma_start(lg_ap[b], lg[:])

        # Software pipeline: block b's counts/mode are emitted after block
        # b+1's scores/threshold/compare, so the PE's counts work overlaps the
        # DVE threshold tail of the next block.
        pending = None
        for b in range(QB):
            # ---- scores S[q, r] = q.r - bb_r/2 for this 128-query block
            # (rank-equivalent to 2*q.r - bb_r; bb folded into the matmul) ----
            S = sbig.tile([P, R], F32, name="S", tag="S")
            for j in range(RCH):
                g, go = (j * 512) // GROUP, (j * 512) % GROUP
                ps = psS_p.tile([P, 512], F32, name="psS", tag="psS")
                for d in range(DCH):
                    nc.tensor.matmul(
                        ps[:],
                        qTt[:, d * Q + b * P: d * Q + (b + 1) * P],
                        xTs[d][g][:, go:go + 512],
                        start=(d == 0),
                        stop=False,
                    )
                nc.tensor.matmul(
                    ps[:],
                    ones2[:],
                    bbts[g][:, go:go + 512],
                    start=False,
                    stop=True,
                )
                nc.scalar.copy(S[:, j * 512:(j + 1) * 512], ps[:])
            # ---- threshold t = k-th largest score of the row ----
            if _stages < 2:
                nc.vector.max(out=small.tile([P, 8], F32, name="mdum", tag="mdum"), in_=S[:, 0:512])
                pending = None
                continue
            m1 = small.tile([P, 8], F32, name="m1", tag="m1", bufs=2)
            nc.vector.max(out=m1[:], in_=S[:])
            if k <= 8:
                mt, col = m1, k - 1
            else:
                Ssc = sbig.tile([P, R], F32, name="Ssc", tag="Ssc", bufs=1)
                nc.vector.match_replace(
                    out=Ssc[:], in_to_replace=m1[:], in_values=S[:], imm_value=-1e30
                )
                m2 = small.tile([P, 8], F32, name="m2", tag="m2", bufs=2)
                nc.vector.max(out=m2[:], in_=Ssc[:])
                mt, col = m2, k - 9
            # ---- mask[q, r] = S >= t_q, in two halves for finer overlap ----
            halves = []
            for h in range(2):
                mh = maskp.tile([P, R2], BF16, name="mh", tag="mh", bufs=3)
                nc.vector.tensor_scalar(
                    out=mh[:], in0=S[:, h * R2:(h + 1) * R2],
                    scalar1=mt[:, col:col + 1], scalar2=None, op0=ALU.is_ge,
                )
                halves.append(mh)
            if _stages >= 3 and pending is not None:
                emit_counts(*pending)
            pending = (b, halves)
        if _stages >= 3:
            emit_counts(*pending)
    nc.compile()
    return nc


# ---------------- host-side packing helpers ----------------

def pack_T(m):
    """[R, D] fp32 -> bf16 [P, (D//P)*R]: column block d holds rows d*P..(d+1)*P
    of m.T (i.e. element (p, d*R + r) = m[r, d*P + p])."""
    R, D = m.shape
    DCH = D // P
    mt = np.ascontiguousarray(m.T.astype(BF16_NP))  # [D, R]
    return np.ascontiguousarray(
        mt.reshape(DCH, P, R).transpose(1, 0, 2).reshape(P, DCH * R)
    )


def pack_bbhl(bb):
    """[R] fp32 -> [2, R] bf16 hi/lo split of -bb/2 (exact to ~2^-17 rel)."""
    t = (-0.5 * bb).astype(np.float32)
    hi = t.astype(BF16_NP)
    lo = (t - hi.astype(np.float32)).astype(BF16_NP)
    return np.ascontiguousarray(np.stack([hi, lo]))


def pack_cols(v):
    """[Q] -> [P, Q//P] fp32: column b = v[b*P:(b+1)*P]."""
    QB = v.shape[0] // P
    return np.ascontiguousarray(v.reshape(QB, P).T.astype(np.float32))


_PROGRAMS = {}
LAST_EXEC_NS = None
_EXEC_NS = {}


def _get_program(key, builder):
    if key not in _PROGRAMS:
        _PROGRAMS[key] = builder()
    return _PROGRAMS[key]


def _run(nc, in_maps, phase):
    import os

    kwargs = {}
    if os.environ.get("KERNEL_TRACE"):
        kwargs = dict(trace=True, trace_cores=[0])
    t0 = _time.time()
    res = run_bass_kernel_spmd(
        nc, in_maps, core_ids=list(range(NCORES)), **kwargs
    )
    if os.environ.get("KERNEL_TIME"):
        print(f"phase {phase} dispatch+exec: {_time.time() - t0:.3f}s")
    if res.exec_time_ns:
        _EXEC_NS[phase] = res.exec_time_ns
        if res.instructions_and_trace:
            print(f"phase {phase}: {res.exec_time_ns} ns, "
                  f"trace: {res.instructions_and_trace[1]}")
    global LAST_EXEC_NS
    if len(_EXEC_NS) == 2:
        LAST_EXEC_NS = sum(_EXEC_NS.values())
    return res


def kernel(x, y, lam, perm):
    x = np.asarray(x, dtype=np.float32)
    y = np.asarray(y, dtype=np.float32)
    lam = np.float32(np.asarray(lam))
    perm = np.asarray(perm, dtype=np.int32)
    N, D = x.shape
    C = CLASSES
    x_ul = (x * lam + x[perm] * (np.float32(1.0) - lam)).astype(np.float32)

    iota_in = np.ascontiguousarray(
        np.broadcast_to(np.arange(C, dtype=np.float32), (P, C))
    )

    # ---------------- phase 1: pseudo-labels via 11-NN mode ----------------
    QA = N // NCORES
    ncA = _get_program(
        ("A", N, QA, D), lambda: build_program(N, QA, D, C, 11, False, False)
    )
    xT_in = pack_T(x)
    bb_x = (x.astype(np.float64) ** 2).sum(1).astype(np.float32)
    bb_in = pack_bbhl(bb_x)
    ylab_in = pack_cols(y)
    in_maps = []
    for c in range(NCORES):
        in_maps.append(
            {
                "xT": xT_in,
                "qT": pack_T(x_ul[c * QA:(c + 1) * QA]),
                "bbhl": bb_in,
                "ylab": ylab_in,
                "iotaf": iota_in,
            }
        )
    # Phase B's big packings depend only on x/x_ul (not on phase A's labels):
    # overlap them with phase A's transfer+execution in a background thread.
    import threading

    xc = np.concatenate([x, x_ul], axis=0)
    _bg = {}

    def _pack_b():
        _bg["xcT"] = pack_T(xc)
        _bg["qTs"] = [
            pack_T(xc[c * (2 * N) // NCORES:(c + 1) * (2 * N) // NCORES])
            for c in range(NCORES)
        ]
        aa_ = (xc.astype(np.float64) ** 2).sum(1).astype(np.float32)
        _bg["aa"] = aa_
        _bg["bbhl2"] = pack_bbhl(aa_)

    _th = threading.Thread(target=_pack_b)
    _th.start()
    resA = _run(ncA, in_maps, "A")
    _th.join()
    y_ul = np.concatenate(
        [r["ymode"].reshape(QA) for r in resA.results]
    ).astype(np.float32)

    # ---------------- host glue: per-class means ----------------
    yc = np.concatenate([y, y_ul], axis=0)
    num = xc.shape[0]
    yi = yc.astype(np.int32)
    counts = np.bincount(yi, minlength=C).astype(np.float32)
    mu = np.zeros((C, D), dtype=np.float32)
    np.add.at(mu, yi, xc)
    mu = mu / np.maximum(counts, 1.0)[:, None]
    bbm = (mu.astype(np.float64) ** 2).sum(1)
    emu = (np.exp(-bbm / 2.0) * (counts > 0)).astype(np.float32)
    emu_in = np.ascontiguousarray(np.broadcast_to(emu, (P, C)))
    aa = _bg["aa"]
    bb_in2 = _bg["bbhl2"]
    ylab2_in = pack_cols(yc)
    muT_in = pack_T(mu)
    xcT_in = _bg["xcT"]

    # ---------------- phase 2: 3-NN mode + gm loss rows ----------------
    QB_ = num // NCORES
    ncB = _get_program(
        ("B", num, QB_, D), lambda: build_program(num, QB_, D, C, 4, True, True)
    )
    in_maps = []
    for c in range(NCORES):
        sl = slice(c * QB_, (c + 1) * QB_)
        qaux = np.concatenate(
            [pack_cols(yc[sl]), pack_cols(-0.5 * aa[sl])], axis=1
        ).astype(np.float32)
        in_maps.append(
            {
                "xT": xcT_in,
                "qT": _bg["qTs"][c],
                "bbhl": bb_in2,
                "ylab": ylab2_in,
                "iotaf": iota_in,
                "qaux": np.ascontiguousarray(qaux),
                "muT": muT_in,
                "emu": emu_in,
            }
        )
    resB = _run(ncB, in_maps, "B")
    y_ng = np.concatenate(
        [r["ymode"].reshape(QB_) for r in resB.results]
    ).astype(np.float32)
    lgm_rows = np.concatenate([r["lgm"].reshape(QB_) for r in resB.results])

    loss_gm = np.float32(lgm_rows.mean(dtype=np.float64))
    loss_knn = np.float32(((y_ng - yc) ** 2).mean(dtype=np.float64))
    return np.float32(loss_gm + np.float32(0.01) * loss_knn)



# revision 3
# speedup vs baseline: 1.1145x; 1.1145x over previous
"""Trainium2 Bass kernel for nn_DGMMLoss (retrieval_knn).

Reference computation (see problem statement):
  1. x_ul = lam*x + (1-lam)*x[perm]; pseudo-label via mode of 11-NN labels
  2. concat; per-class means; gaussian-mixture loss term
  3. kNN regularizer: mode of 3-NN (self-excluded) labels, MSE
  loss = loss_gm + 0.01 * loss_knn

Device strategy (8 NeuronCores, data-parallel over query rows; two SPMD
launches):
  - Scores s[q,r] = q.r - ||r||^2/2 via bf16 matmuls (fp32 psum); the -bb/2
    term rides in the same accumulation as an augmented K=2 contraction of a
    ones column against a bf16 hi/lo split of -bb/2 (exact to ~2^-17 rel),
    so psum evacuation is a pure copy and runs on the ACT engine.
  - Phase A (11-NN pseudo-labels): per-row k-th largest via DVE max8
    (+match_replace+max8 for k=11) gives a per-partition threshold; one
    tensor_scalar is_ge produces the bf16 mask[q,r] per 128-query block.
    Per-class counts = maskT.T @ onehot(y) on the PE (mask tiles transposed
    on the PE via identity, batched per PSUM bank, evacuated by ACT copies);
    mode = first argmax of counts (smallest class on ties, matching
    torch.mode) via reduce_max / is_lt / reduce_min on DVE. Blocks are
    software-pipelined so PE counts work overlaps the DVE threshold tail.
  - Phase B (3-NN + gm rows): instead of masks/counts, the top-8 scores per
    row come from two half-row max8s merged by a third max8; one DVE
    max_index scan yields their uint16 column indices, which are DMA'd to
    the host. Self is always rank-0 (score gap ~2 orders above bf16 noise),
    so host-side mode over gathered labels of ranks 1..3 reproduces the
    self-excluded 3-NN mode. This removes all PE transposes/counts matmuls
    and ACT mask copies of the old design; per-block device work is score
    matmuls (PE), psum evacuation (ACT), and 3 max8 + 1 max_index scans
    (DVE), pipelined across blocks.
  - GM branch (phase B, on device): pi = exp(q.mu - aa/2)*exp(-||mu||^2/2)
    *(counts>0), row-normalized; per-row sum((pi - onehot)^2).
Host does only O(N*D) glue: x_ul, norms, packing, per-class means,
label-gather + mode-of-3 from device indices, final scalar assembly. bf16
scoring shifts the loss by ~9e-4 relative (verified against an fp64 model;
fp32 matmul on TRN2 is 4x slower than bf16).
"""

from contextlib import ExitStack

import numpy as np
import ml_dtypes

import time as _time

import concourse.bacc as bacc
import concourse.tile as tile
import concourse.mybir as mybir
from concourse.bass_utils import run_bass_kernel_spmd
from concourse.masks import make_identity

P = 128
NCORES = 8
CLASSES = 100
F32 = mybir.dt.float32
BF16 = mybir.dt.bfloat16
U16 = mybir.dt.uint16
BF16_NP = ml_dtypes.bfloat16
ALU = mybir.AluOpType
AX = mybir.AxisListType


def build_program(R, Q, D, C, k, self_exclude, gm, n_cores=NCORES, _stages=3):
    """Phase A pipeline as a Bass/Tile program (SPMD over cores).

    R: number of reference rows (shared across cores)
    Q: number of query rows handled by this core
    k: keep the k nearest (largest score) refs per query row
    self_exclude: subtract the query's own label from the counts (knn branch)
    gm: also compute the per-row gaussian-mixture loss term
    """
    DCH, RT, RCH, QB = D // P, R // P, R // 512, Q // P
    assert D % P == 0 and R % 512 == 0 and Q % P == 0 and k <= 16

    nc = bacc.Bacc(
        "TRN2", target_bir_lowering=False, debug=False, num_devices=n_cores
    )
    xT_ap = nc.dram_tensor("xT", [P, DCH * R], BF16, kind="ExternalInput").ap()
    qT_ap = nc.dram_tensor("qT", [P, DCH * Q], BF16, kind="ExternalInput").ap()
    # -||r||^2/2 split into bf16 hi+lo rows, folded into the score matmul as
    # an augmented K=2 contraction against a column of ones.
    bb_ap = nc.dram_tensor("bbhl", [2, R], BF16, kind="ExternalInput").ap()
    # reference labels packed [P, RT]: column i holds y[i*128 : (i+1)*128]
    yl_ap = nc.dram_tensor("ylab", [P, RT], F32, kind="ExternalInput").ap()
    io_ap = nc.dram_tensor("iotaf", [P, C], F32, kind="ExternalInput").ap()
    nqaux = (2 * QB) if gm else QB
    qaux_ap = (
        nc.dram_tensor("qaux", [P, nqaux], F32, kind="ExternalInput").ap()
        if (self_exclude or gm)
        else None
    )
    muT_ap = emu_ap = None
    if gm:
        muT_ap = nc.dram_tensor("muT", [P, DCH * C], BF16, kind="ExternalInput").ap()
        emu_ap = nc.dram_tensor("emu", [P, C], F32, kind="ExternalInput").ap()
    ym_ap = nc.dram_tensor("ymode", [QB, P, 1], F32, kind="ExternalOutput").ap()
    lg_ap = (
        nc.dram_tensor("lgm", [QB, P, 1], F32, kind="ExternalOutput").ap()
        if gm
        else None
    )

    with tile.TileContext(nc) as tc, ExitStack() as ctx:
        consts = ctx.enter_context(tc.tile_pool(name="consts", bufs=1))
        sbig = ctx.enter_context(tc.tile_pool(name="sbig", bufs=2))
        maskp = ctx.enter_context(tc.tile_pool(name="maskp", bufs=1))
        small = ctx.enter_context(tc.tile_pool(name="small", bufs=1))
        psS_p = ctx.enter_context(tc.tile_pool(name="psS", bufs=3, space="PSUM"))
        psT_p = ctx.enter_context(tc.tile_pool(name="psT", bufs=2, space="PSUM"))
        psC_p = ctx.enter_context(tc.tile_pool(name="psC", bufs=1, space="PSUM"))
        psG_p = (
            ctx.enter_context(tc.tile_pool(name="psG", bufs=1, space="PSUM"))
            if gm
            else None
        )

        identb = consts.tile([P, P], BF16, name="identb", tag="identb")
        make_identity(nc, identb)

        # Tiny "touch" ops absorb DMA-queue waits into dedicated copies so the
        # wide compute instructions (1-2 HW wait slots) only wait on engine
        # semaphores.
        tchV = consts.tile([1, 1], F32, name="tchV", tag="tchV")
        tchA = consts.tile([1, 1], F32, name="tchA", tag="tchA")

        def dve_touch(ap):
            nc.vector.tensor_copy(tchV[:], ap[0:1, 0:1])

        def act_touch(ap):
            nc.scalar.copy(tchA[:], ap[0:1, 0:1])

        # PE touch of the identity so later transposes don't carry its wait.
        psI = psT_p.tile([1, P], BF16, name="psI", tag="psMI", bufs=1)
        nc.tensor.transpose(psI[:], identb[:, 0:1], identb[:])

        # DMA constants in. One dma_start per tile (Tile deps are per-tile, and
        # matmuls only have ~2 wait slots); big constants are split into
        # separate ref-group tiles so compute can start after the first group.
        GROUP = min(R, 1024)
        NG = R // GROUP
        xTs = [[None] * NG for _ in range(DCH)]
        for g in range(NG):
            for d in range(DCH):
                t = consts.tile(
                    [P, GROUP], BF16, name=f"xTs{d}_{g}", tag=f"xTs{d}_{g}"
                )
                nc.sync.dma_start(
                    t[:], xT_ap[:, d * R + g * GROUP: d * R + (g + 1) * GROUP]
                )
                xTs[d][g] = t
        qTt = consts.tile([P, DCH * Q], BF16, name="qTt", tag="qTt")
        nc.sync.dma_start(qTt[:], qT_ap[:])
        ones2 = consts.tile([2, P], BF16, name="ones2", tag="ones2")
        nc.vector.memset(ones2[:], 1.0)
        bbts = []
        for g in range(NG):
            t = consts.tile([2, GROUP], BF16, name=f"bbt{g}", tag=f"bbt{g}")
            nc.sync.dma_start(t[:], bb_ap[:, g * GROUP:(g + 1) * GROUP])
            bbts.append(t)
        ylabt = consts.tile([P, RT], F32, name="ylabt", tag="ylabt")
        nc.sync.dma_start(ylabt[:], yl_ap[:])
        iot = consts.tile([P, C], F32, name="iot", tag="iot")
        nc.sync.dma_start(iot[:], io_ap[:])
        if qaux_ap is not None:
            qauxt = consts.tile([P, nqaux], F32, name="qauxt", tag="qauxt")
            nc.sync.dma_start(qauxt[:], qaux_ap[:])
        if gm:
            muTt = consts.tile([P, DCH * C], BF16, name="muTt", tag="muTt")
            nc.sync.dma_start(muTt[:], muT_ap[:])
            emut = consts.tile([P, C], F32, name="emut", tag="emut")
            nc.sync.dma_start(emut[:], emu_ap[:])
        dve_touch(iot)
        dve_touch(ylabt)
        # one-hot labels built on device: yoht[:, i*C:(i+1)*C] = (iota == y_r)
        yoht = consts.tile([P, RT * C], BF16, name="yoht", tag="yoht")
        for i in range(RT):
            nc.vector.tensor_scalar(
                out=yoht[:, i * C:(i + 1) * C], in0=iot[:],
                scalar1=ylabt[:, i:i + 1], scalar2=None, op0=ALU.is_equal,
            )
        if qaux_ap is not None:
            dve_touch(qauxt)
            act_touch(qauxt)
        if gm:
            dve_touch(emut)

        R2 = R // 2
        HT = RT // 2  # mask tiles per half

        def emit_counts(b, halves):
            """Counts + mode (+ gm) for query block b given its mask halves."""
            psc = psC_p.tile([P, C], F32, name="psC", tag="psC")
            GT = min(8, RT)  # transposes batched per PSUM bank / ACT copy
            for i0 in range(0, RT, GT):
                pst = psT_p.tile([P, GT * P], BF16, name="psT", tag="psT")
                for u in range(GT):
                    i = i0 + u
                    mh = halves[i // HT]
                    lo = (i % HT) * P
                    nc.tensor.transpose(
                        pst[:, u * P:(u + 1) * P], mh[:, lo:lo + P], identb[:]
                    )
                mTg = maskp.tile([P, GT * P], BF16, name="mTg", tag="mTg", bufs=3)
                nc.scalar.copy(mTg[:], pst[:])
                for u in range(GT):
                    i = i0 + u
                    nc.tensor.matmul(
                        psc[:],
                        mTg[:, u * P:(u + 1) * P],
                        yoht[:, i * C:(i + 1) * C],
                        start=(i == 0),
                        stop=(i == RT - 1),
                    )
            counts = small.tile([P, C], F32, name="counts", tag="counts")
            if self_exclude or gm:
                yh = small.tile([P, C], F32, name="yh", tag="yh")
                nc.vector.tensor_scalar(
                    out=yh[:],
                    in0=iot[:],
                    scalar1=qauxt[:, b:b + 1],
                    scalar2=None,
                    op0=ALU.is_equal,
                )
            if self_exclude:
                nc.vector.tensor_sub(counts[:], psc[:], yh[:])
            else:
                nc.vector.tensor_copy(counts[:], psc[:])
            # mode = first argmax of counts
            maxc = small.tile([P, 1], F32, name="maxc", tag="maxc")
            nc.vector.reduce_max(maxc[:], counts[:], axis=AX.X)
            lt01 = small.tile([P, C], F32, name="lt01", tag="lt01")
            nc.vector.tensor_scalar(
                out=lt01[:], in0=counts[:], scalar1=maxc[:], scalar2=None,
                op0=ALU.is_lt,
            )
            cand = small.tile([P, C], F32, name="cand", tag="cand")
            nc.vector.scalar_tensor_tensor(
                out=cand[:], in0=lt01[:], scalar=1e9, in1=iot[:],
                op0=ALU.mult, op1=ALU.add,
            )
            ym = small.tile([P, 1], F32, name="ym", tag="ym")
            nc.vector.tensor_reduce(ym[:], cand[:], axis=AX.X, op=ALU.min)
            nc.sync.dma_start(ym_ap[b], ym[:])
            # gaussian-mixture per-row loss
            if gm:
                psg = psG_p.tile([P, C], F32, name="psG", tag="psG")
                for d in range(DCH):
                    nc.tensor.matmul(
                        psg[:],
                        qTt[:, d * Q + b * P: d * Q + (b + 1) * P],
                        muTt[:, d * C:(d + 1) * C],
                        start=(d == 0),
                        stop=(d == DCH - 1),
                    )
                eg = small.tile([P, C], F32, name="eg", tag="eg")
                nc.scalar.activation(
                    eg[:], psg[:], mybir.ActivationFunctionType.Exp,
                    bias=qauxt[:, QB + b:QB + b + 1], scale=1.0,
                )
                piu = small.tile([P, C], F32, name="piu", tag="piu")
                nc.vector.tensor_mul(piu[:], eg[:], emut[:])
                srow = small.tile([P, 1], F32, name="srow", tag="srow")
                nc.vector.reduce_sum(srow[:], piu[:], axis=AX.X)
                nc.vector.tensor_scalar_add(srow[:], srow[:], 1e-15)
                rec = small.tile([P, 1], F32, name="rec", tag="rec")
                nc.vector.reciprocal(rec[:], srow[:])
                pin = small.tile([P, C], F32, name="pin", tag="pin")
                nc.vector.tensor_scalar(
                    out=pin[:], in0=piu[:], scalar1=rec[:], scalar2=None,
                    op0=ALU.mult,
                )
                diff = small.tile([P, C], F32, name="diff", tag="diff")
                nc.vector.tensor_sub(diff[:], pin[:], yh[:])
                sq = small.tile([P, C], F32, name="sq", tag="sq")
                nc.vector.tensor_mul(sq[:], diff[:], diff[:])
                lg = small.tile([P, 1], F32, name="lg", tag="lg")
                nc.vector.reduce_sum(lg[:], sq[:], axis=AX.X)
                nc.sync.dma_start(lg_ap[b], lg[:])

        # Software pipeline: block b's counts/mode are emitted after block
        # b+1's scores/threshold/compare, so the PE's counts work overlaps the
        # DVE threshold tail of the next block.
        pending = None
        for b in range(QB):
            # ---- scores S[q, r] = q.r - bb_r/2 for this 128-query block
            # (rank-equivalent to 2*q.r - bb_r; bb folded into the matmul) ----
            S = sbig.tile([P, R], F32, name="S", tag="S")
            for j in range(RCH):
                g, go = (j * 512) // GROUP, (j * 512) % GROUP
                ps = psS_p.tile([P, 512], F32, name="psS", tag="psS")
                for d in range(DCH):
                    nc.tensor.matmul(
                        ps[:],
                        qTt[:, d * Q + b * P: d * Q + (b + 1) * P],
                        xTs[d][g][:, go:go + 512],
                        start=(d == 0),
                        stop=False,
                    )
                nc.tensor.matmul(
                    ps[:],
                    ones2[:],
                    bbts[g][:, go:go + 512],
                    start=False,
                    stop=True,
                )
                nc.scalar.copy(S[:, j * 512:(j + 1) * 512], ps[:])
            # ---- threshold t = k-th largest score of the row ----
            if _stages < 2:
                nc.vector.max(out=small.tile([P, 8], F32, name="mdum", tag="mdum"), in_=S[:, 0:512])
                pending = None
                continue
            m1 = small.tile([P, 8], F32, name="m1", tag="m1", bufs=2)
            nc.vector.max(out=m1[:], in_=S[:])
            if k <= 8:
                mt, col = m1, k - 1
            else:
                Ssc = sbig.tile([P, R], F32, name="Ssc", tag="Ssc", bufs=1)
                nc.vector.match_replace(
                    out=Ssc[:], in_to_replace=m1[:], in_values=S[:], imm_value=-1e30
                )
                m2 = small.tile([P, 8], F32, name="m2", tag="m2", bufs=2)
                nc.vector.max(out=m2[:], in_=Ssc[:])
                mt, col = m2, k - 9
            # ---- mask[q, r] = S >= t_q, in two halves for finer overlap ----
            halves = []
            for h in range(2):
                mh = maskp.tile([P, R2], BF16, name="mh", tag="mh", bufs=3)
                nc.vector.tensor_scalar(
                    out=mh[:], in0=S[:, h * R2:(h + 1) * R2],
                    scalar1=mt[:, col:col + 1], scalar2=None, op0=ALU.is_ge,
                )
                halves.append(mh)
            if _stages >= 3 and pending is not None:
                emit_counts(*pending)
            pending = (b, halves)
        if _stages >= 3:
            emit_counts(*pending)
    nc.compile()
    return nc


def build_program_b(R, Q, D, C, n_cores=NCORES):
    """Phase B: top-8 neighbor indices per row (uint16, host decodes the
    3-NN mode) + per-row gaussian-mixture loss, SPMD over cores."""
    DCH, RCH, QB = D // P, R // 512, Q // P
    assert D % P == 0 and R % 512 == 0 and Q % P == 0

    nc = bacc.Bacc(
        "TRN2", target_bir_lowering=False, debug=False, num_devices=n_cores
    )
    xT_ap = nc.dram_tensor("xT", [P, DCH * R], BF16, kind="ExternalInput").ap()
    qT_ap = nc.dram_tensor("qT", [P, DCH * Q], BF16, kind="ExternalInput").ap()
    bb_ap = nc.dram_tensor("bbhl", [2, R], BF16, kind="ExternalInput").ap()
    io_ap = nc.dram_tensor("iotaf", [P, C], F32, kind="ExternalInput").ap()
    # qaux col b = own labels of block b; col QB+b = -aa/2 (exp bias)
    qaux_ap = nc.dram_tensor("qaux", [P, 2 * QB], F32, kind="ExternalInput").ap()
    muT_ap = nc.dram_tensor("muT", [P, DCH * C], BF16, kind="ExternalInput").ap()
    emu_ap = nc.dram_tensor("emu", [P, C], F32, kind="ExternalInput").ap()
    idx_ap = nc.dram_tensor("idxo", [QB, P, 8], U16, kind="ExternalOutput").ap()
    lg_ap = nc.dram_tensor("lgm", [QB, P, 1], F32, kind="ExternalOutput").ap()

    with tile.TileContext(nc) as tc, ExitStack() as ctx:
        consts = ctx.enter_context(tc.tile_pool(name="consts", bufs=1))
        sbig = ctx.enter_context(tc.tile_pool(name="sbig", bufs=2))
        small = ctx.enter_context(tc.tile_pool(name="small", bufs=1))
        psS_p = ctx.enter_context(tc.tile_pool(name="psS", bufs=3, space="PSUM"))
        psG_p = ctx.enter_context(tc.tile_pool(name="psG", bufs=2, space="PSUM"))

        tchV = consts.tile([1, 1], F32, name="tchV", tag="tchV")
        tchA = consts.tile([1, 1], F32, name="tchA", tag="tchA")

        def dve_touch(ap):
            nc.vector.tensor_copy(tchV[:], ap[0:1, 0:1])

        def act_touch(ap):
            nc.scalar.copy(tchA[:], ap[0:1, 0:1])

        GROUP = min(R, 1024)
        NG = R // GROUP
        xTs = [[None] * NG for _ in range(DCH)]
        for g in range(NG):
            for d in range(DCH):
                t = consts.tile(
                    [P, GROUP], BF16, name=f"xTs{d}_{g}", tag=f"xTs{d}_{g}"
                )
                nc.sync.dma_start(
                    t[:], xT_ap[:, d * R + g * GROUP: d * R + (g + 1) * GROUP]
                )
                xTs[d][g] = t
        qTt = consts.tile([P, DCH * Q], BF16, name="qTt", tag="qTt")
        nc.sync.dma_start(qTt[:], qT_ap[:])
        ones2 = consts.tile([2, P], BF16, name="ones2", tag="ones2")
        nc.vector.memset(ones2[:], 1.0)
        bbts = []
        for g in range(NG):
            t = consts.tile([2, GROUP], BF16, name=f"bbt{g}", tag=f"bbt{g}")
            nc.sync.dma_start(t[:], bb_ap[:, g * GROUP:(g + 1) * GROUP])
            bbts.append(t)
        iot = consts.tile([P, C], F32, name="iot", tag="iot")
        nc.sync.dma_start(iot[:], io_ap[:])
        qauxt = consts.tile([P, 2 * QB], F32, name="qauxt", tag="qauxt")
        nc.sync.dma_start(qauxt[:], qaux_ap[:])
        muTt = consts.tile([P, DCH * C], BF16, name="muTt", tag="muTt")
        nc.sync.dma_start(muTt[:], muT_ap[:])
        emut = consts.tile([P, C], F32, name="emut", tag="emut")
        nc.sync.dma_start(emut[:], emu_ap[:])
        dve_touch(iot)
        dve_touch(qauxt)
        act_touch(qauxt)
        dve_touch(emut)

        R2 = R // 2
        for b in range(QB):
            # ---- scores S[q, r] = q.r - bb_r/2 for this 128-query block ----
            S = sbig.tile([P, R], F32, name="S", tag="S")
            for j in range(RCH):
                g, go = (j * 512) // GROUP, (j * 512) % GROUP
                ps = psS_p.tile([P, 512], F32, name="psS", tag="psS")
                for d in range(DCH):
                    nc.tensor.matmul(
                        ps[:],
                        qTt[:, d * Q + b * P: d * Q + (b + 1) * P],
                        xTs[d][g][:, go:go + 512],
                        start=(d == 0),
                        stop=False,
                    )
                nc.tensor.matmul(
                    ps[:],
                    ones2[:],
                    bbts[g][:, go:go + 512],
                    start=False,
                    stop=True,
                )
                nc.scalar.copy(S[:, j * 512:(j + 1) * 512], ps[:])
            # ---- gm matmuls ride behind the scores on the PE ----
            psg = psG_p.tile([P, C], F32, name="psG", tag="psG")
            for d in range(DCH):
                nc.tensor.matmul(
                    psg[:],
                    qTt[:, d * Q + b * P: d * Q + (b + 1) * P],
                    muTt[:, d * C:(d + 1) * C],
                    start=(d == 0),
                    stop=(d == DCH - 1),
                )
            eg = small.tile([P, C], F32, name="eg", tag="eg", bufs=2)
            nc.scalar.activation(
                eg[:], psg[:], mybir.ActivationFunctionType.Exp,
                bias=qauxt[:, QB + b:QB + b + 1], scale=1.0,
            )
            # ---- top-8: two half-row max8s merged, one index scan ----
            m16 = small.tile([P, 16], F32, name="m16", tag="m16", bufs=2)
            nc.vector.max(out=m16[:, 0:8], in_=S[:, 0:R2])
            nc.vector.max(out=m16[:, 8:16], in_=S[:, R2:R])
            m8 = small.tile([P, 8], F32, name="m8", tag="m8", bufs=2)
            nc.vector.max(out=m8[:], in_=m16[:])
            idx16 = small.tile([P, 8], U16, name="idx16", tag="idx16", bufs=2)
            nc.vector.max_index(idx16[:], m8[:], S[:])
            nc.sync.dma_start(idx_ap[b], idx16[:])
            # ---- gaussian-mixture per-row loss ----
            yh = small.tile([P, C], F32, name="yh", tag="yh")
            nc.vector.tensor_scalar(
                out=yh[:], in0=iot[:], scalar1=qauxt[:, b:b + 1],
                scalar2=None, op0=ALU.is_equal,
            )
            piu = small.tile([P, C], F32, name="piu", tag="piu")
            nc.vector.tensor_mul(piu[:], eg[:], emut[:])
            srow = small.tile([P, 1], F32, name="srow", tag="srow")
            nc.vector.reduce_sum(srow[:], piu[:], axis=AX.X)
            nc.vector.tensor_scalar_add(srow[:], srow[:], 1e-15)
            rec = small.tile([P, 1], F32, name="rec", tag="rec")
            nc.vector.reciprocal(rec[:], srow[:])
            pin = small.tile([P, C], F32, name="pin", tag="pin")
            nc.vector.tensor_scalar(
                out=pin[:], in0=piu[:], scalar1=rec[:], scalar2=None,
                op0=ALU.mult,
            )
            diff = small.tile([P, C], F32, name="diff", tag="diff")
            nc.vector.tensor_sub(diff[:], pin[:], yh[:])
            sq = small.tile([P, C], F32, name="sq", tag="sq")
            nc.vector.tensor_mul(sq[:], diff[:], diff[:])
            lg = small.tile([P, 1], F32, name="lg", tag="lg")
            nc.vector.reduce_sum(lg[:], sq[:], axis=AX.X)
            nc.sync.dma_start(lg_ap[b], lg[:])
    nc.compile()
    return nc


# ---------------- host-side packing helpers ----------------

def pack_T(m):
    """[R, D] fp32 -> bf16 [P, (D//P)*R]: column block d holds rows d*P..(d+1)*P
    of m.T (i.e. element (p, d*R + r) = m[r, d*P + p])."""
    R, D = m.shape
    DCH = D // P
    mt = np.ascontiguousarray(m.T.astype(BF16_NP))  # [D, R]
    return np.ascontiguousarray(
        mt.reshape(DCH, P, R).transpose(1, 0, 2).reshape(P, DCH * R)
    )


def pack_bbhl(bb):
    """[R] fp32 -> [2, R] bf16 hi/lo split of -bb/2 (exact to ~2^-17 rel)."""
    t = (-0.5 * bb).astype(np.float32)
    hi = t.astype(BF16_NP)
    lo = (t - hi.astype(np.float32)).astype(BF16_NP)
    return np.ascontiguousarray(np.stack([hi, lo]))


def pack_cols(v):
    """[Q] -> [P, Q//P] fp32: column b = v[b*P:(b+1)*P]."""
    QB = v.shape[0] // P
    return np.ascontiguousarray(v.reshape(QB, P).T.astype(np.float32))


def mode_rows_host(vals):
    """[M, K] labels -> [M] torch.mode semantics (most frequent, smallest on
    ties)."""
    eq = vals[:, :, None] == vals[:, None, :]
    counts = eq.sum(axis=2)
    maxc = counts.max(axis=1, keepdims=True)
    masked = np.where(counts == maxc, vals, np.inf)
    return masked.min(axis=1)


_PROGRAMS = {}
LAST_EXEC_NS = None
_EXEC_NS = {}


def _get_program(key, builder):
    if key not in _PROGRAMS:
        _PROGRAMS[key] = builder()
    return _PROGRAMS[key]


def _run(nc, in_maps, phase):
    import os

    kwargs = {}
    if os.environ.get("KERNEL_TRACE"):
        kwargs = dict(trace=True, trace_cores=[0])
    t0 = _time.time()
    res = run_bass_kernel_spmd(
        nc, in_maps, core_ids=list(range(NCORES)), **kwargs
    )
    if os.environ.get("KERNEL_TIME"):
        print(f"phase {phase} dispatch+exec: {_time.time() - t0:.3f}s")
    if res.exec_time_ns:
        _EXEC_NS[phase] = res.exec_time_ns
        if res.instructions_and_trace:
            print(f"phase {phase}: {res.exec_time_ns} ns, "
                  f"trace: {res.instructions_and_trace[1]}")
    global LAST_EXEC_NS
    if len(_EXEC_NS) == 2:
        LAST_EXEC_NS = sum(_EXEC_NS.values())
    return res


def kernel(x, y, lam, perm):
    x = np.asarray(x, dtype=np.float32)
    y = np.asarray(y, dtype=np.float32)
    lam = np.float32(np.asarray(lam))
    perm = np.asarray(perm, dtype=np.int32)
    N, D = x.shape
    C = CLASSES
    x_ul = (x * lam + x[perm] * (np.float32(1.0) - lam)).astype(np.float32)

    iota_in = np.ascontiguousarray(
        np.broadcast_to(np.arange(C, dtype=np.float32), (P, C))
    )

    # ---------------- phase 1: pseudo-labels via 11-NN mode ----------------
    QA = N // NCORES
    ncA = _get_program(
        ("A", N, QA, D), lambda: build_program(N, QA, D, C, 11, False, False)
    )
    xT_in = pack_T(x)
    bb_x = (x.astype(np.float64) ** 2).sum(1).astype(np.float32)
    bb_in = pack_bbhl(bb_x)
    ylab_in = pack_cols(y)
    in_maps = []
    for c in range(NCORES):
        in_maps.append(
            {
                "xT": xT_in,
                "qT": pack_T(x_ul[c * QA:(c + 1) * QA]),
                "bbhl": bb_in,
                "ylab": ylab_in,
                "iotaf": iota_in,
            }
        )
    # Phase B's big packings depend only on x/x_ul (not on phase A's labels):
    # overlap them with phase A's transfer+execution in a background thread.
    import threading

    xc = np.concatenate([x, x_ul], axis=0)
    _bg = {}

    def _pack_b():
        _bg["xcT"] = pack_T(xc)
        _bg["qTs"] = [
            pack_T(xc[c * (2 * N) // NCORES:(c + 1) * (2 * N) // NCORES])
            for c in range(NCORES)
        ]
        aa_ = (xc.astype(np.float64) ** 2).sum(1).astype(np.float32)
        _bg["aa"] = aa_
        _bg["bbhl2"] = pack_bbhl(aa_)

    _th = threading.Thread(target=_pack_b)
    _th.start()
    resA = _run(ncA, in_maps, "A")
    _th.join()
    y_ul = np.concatenate(
        [r["ymode"].reshape(QA) for r in resA.results]
    ).astype(np.float32)

    # ---------------- host glue: per-class means ----------------
    yc = np.concatenate([y, y_ul], axis=0)
    num = xc.shape[0]
    yi = yc.astype(np.int32)
    counts = np.bincount(yi, minlength=C).astype(np.float32)
    mu = np.zeros((C, D), dtype=np.float32)
    np.add.at(mu, yi, xc)
    mu = mu / np.maximum(counts, 1.0)[:, None]
    bbm = (mu.astype(np.float64) ** 2).sum(1)
    emu = (np.exp(-bbm / 2.0) * (counts > 0)).astype(np.float32)
    emu_in = np.ascontiguousarray(np.broadcast_to(emu, (P, C)))
    aa = _bg["aa"]
    bb_in2 = _bg["bbhl2"]
    muT_in = pack_T(mu)
    xcT_in = _bg["xcT"]

    # ---------------- phase 2: 3-NN indices + gm loss rows ----------------
    QB_ = num // NCORES
    ncB = _get_program(
        ("B", num, QB_, D), lambda: build_program_b(num, QB_, D, C)
    )
    in_maps = []
    for c in range(NCORES):
        sl = slice(c * QB_, (c + 1) * QB_)
        qaux = np.concatenate(
            [pack_cols(yc[sl]), pack_cols(-0.5 * aa[sl])], axis=1
        ).astype(np.float32)
        in_maps.append(
            {
                "xT": xcT_in,
                "qT": _bg["qTs"][c],
                "bbhl": bb_in2,
                "iotaf": iota_in,
                "qaux": np.ascontiguousarray(qaux),
                "muT": muT_in,
                "emu": emu_in,
            }
        )
    resB = _run(ncB, in_maps, "B")
    # idxo[b, p, j] = j-th nearest ref of query (b*128 + p); rank 0 is self.
    idx_all = np.concatenate(
        [r["idxo"].reshape(QB_, 8) for r in resB.results]
    ).astype(np.int64)
    lgm_rows = np.concatenate([r["lgm"].reshape(QB_) for r in resB.results])

    y_ng = mode_rows_host(yc[idx_all[:, 1:4]]).astype(np.float32)

    loss_gm = np.float32(lgm_rows.mean(dtype=np.float64))
    loss_knn = np.float32(((y_ng - yc) ** 2).mean(dtype=np.float64))
    return np.float32(loss_gm + np.float32(0.01) * loss_knn)


# revision 5
# speedup vs baseline: 1.2715x; 1.1409x over previous
"""Trainium2 Bass kernel for nn_DGMMLoss (retrieval_knn).

Reference computation (see problem statement):
  1. x_ul = lam*x + (1-lam)*x[perm]; pseudo-label via mode of 11-NN labels
  2. concat; per-class means; gaussian-mixture loss term
  3. kNN regularizer: mode of 3-NN (self-excluded) labels, MSE
  loss = loss_gm + 0.01 * loss_knn

Device strategy (8 NeuronCores, data-parallel over query rows; two SPMD
launches):
  - Scores s[q,r] = q.r - ||r||^2/2 via bf16 matmuls (fp32 psum); the -bb/2
    term rides in the same accumulation as an augmented K=2 contraction of a
    ones column against a bf16 hi/lo split of -bb/2 (exact to ~2^-17 rel),
    so psum evacuation is a pure copy and runs on the ACT engine.
  - Phase A (11-NN pseudo-labels): per-row k-th largest via DVE max8
    (+match_replace+max8 for k=11) gives a per-partition threshold; one
    tensor_scalar is_ge produces the bf16 mask[q,r] per 128-query block.
    Per-class counts = maskT.T @ onehot(y) on the PE (mask tiles transposed
    on the PE via identity, batched per PSUM bank, evacuated by ACT copies);
    mode = first argmax of counts (smallest class on ties, matching
    torch.mode) via reduce_max / is_lt / reduce_min on DVE. Blocks are
    software-pipelined so PE counts work overlaps the DVE threshold tail.
  - Phase B (3-NN + gm rows): instead of masks/counts, the top-8 scores per
    row come from two half-row max8s merged by a third max8; one DVE
    max_index scan yields their uint16 column indices, which are DMA'd to
    the host. Self is always rank-0 (score gap ~2 orders above bf16 noise),
    so host-side mode over gathered labels of ranks 1..3 reproduces the
    self-excluded 3-NN mode. This removes all PE transposes/counts matmuls
    and ACT mask copies of the old design; per-block device work is score
    matmuls (PE), psum evacuation (ACT), and 3 max8 + 1 max_index scans
    (DVE), pipelined across blocks.
  - GM branch (phase B, on device): pi = exp(q.mu - aa/2)*exp(-||mu||^2/2)
    *(counts>0), row-normalized; per-row sum((pi - onehot)^2).
Host does only O(N*D) glue: x_ul, norms, packing, per-class means,
label-gather + mode-of-3 from device indices, final scalar assembly. bf16
scoring shifts the loss by ~9e-4 relative (verified against an fp64 model;
fp32 matmul on TRN2 is 4x slower than bf16).
"""

from contextlib import ExitStack

import numpy as np
import ml_dtypes

import time as _time

import concourse.bacc as bacc
import concourse.tile as tile
import concourse.mybir as mybir
from concourse.bass_utils import run_bass_kernel_spmd
from concourse.masks import make_identity

P = 128
NCORES = 8
CLASSES = 100
F32 = mybir.dt.float32
BF16 = mybir.dt.bfloat16
U16 = mybir.dt.uint16
BF16_NP = ml_dtypes.bfloat16
ALU = mybir.AluOpType
AX = mybir.AxisListType


def build_program(R, Q, D, C, k, self_exclude, gm, n_cores=NCORES, _stages=3):
    """Phase A pipeline as a Bass/Tile program (SPMD over cores).

    R: number of reference rows (shared across cores)
    Q: number of query rows handled by this core
    k: keep the k nearest (largest score) refs per query row
    self_exclude: subtract the query's own label from the counts (knn branch)
    gm: also compute the per-row gaussian-mixture loss term
    """
    DCH, RT, RCH, QB = D // P, R // P, R // 512, Q // P
    assert D % P == 0 and R % 512 == 0 and Q % P == 0 and k <= 16

    nc = bacc.Bacc(
        "TRN2", target_bir_lowering=False, debug=False, num_devices=n_cores
    )
    xT_ap = nc.dram_tensor("xT", [P, DCH * R], BF16, kind="ExternalInput").ap()
    qT_ap = nc.dram_tensor("qT", [P, DCH * Q], BF16, kind="ExternalInput").ap()
    # -||r||^2/2 split into bf16 hi+lo rows, folded into the score matmul as
    # an augmented K=2 contraction against a column of ones.
    bb_ap = nc.dram_tensor("bbhl", [2, R], BF16, kind="ExternalInput").ap()
    # reference labels packed [P, RT]: column i holds y[i*128 : (i+1)*128]
    yl_ap = nc.dram_tensor("ylab", [P, RT], F32, kind="ExternalInput").ap()
    io_ap = nc.dram_tensor("iotaf", [P, C], F32, kind="ExternalInput").ap()
    nqaux = (2 * QB) if gm else QB
    qaux_ap = (
        nc.dram_tensor("qaux", [P, nqaux], F32, kind="ExternalInput").ap()
        if (self_exclude or gm)
        else None
    )
    muT_ap = emu_ap = None
    if gm:
        muT_ap = nc.dram_tensor("muT", [P, DCH * C], BF16, kind="ExternalInput").ap()
        emu_ap = nc.dram_tensor("emu", [P, C], F32, kind="ExternalInput").ap()
    ym_ap = nc.dram_tensor("ymode", [QB, P, 1], F32, kind="ExternalOutput").ap()
    lg_ap = (
        nc.dram_tensor("lgm", [QB, P, 1], F32, kind="ExternalOutput").ap()
        if gm
        else None
    )

    with tile.TileContext(nc) as tc, ExitStack() as ctx:
        consts = ctx.enter_context(tc.tile_pool(name="consts", bufs=1))
        sbig = ctx.enter_context(tc.tile_pool(name="sbig", bufs=2))
        maskp = ctx.enter_context(tc.tile_pool(name="maskp", bufs=1))
        small = ctx.enter_context(tc.tile_pool(name="small", bufs=1))
        psS_p = ctx.enter_context(tc.tile_pool(name="psS", bufs=3, space="PSUM"))
        psT_p = ctx.enter_context(tc.tile_pool(name="psT", bufs=2, space="PSUM"))
        psC_p = ctx.enter_context(tc.tile_pool(name="psC", bufs=1, space="PSUM"))
        psG_p = (
            ctx.enter_context(tc.tile_pool(name="psG", bufs=1, space="PSUM"))
            if gm
            else None
        )

        identb = consts.tile([P, P], BF16, name="identb", tag="identb")
        make_identity(nc, identb)

        # Tiny "touch" ops absorb DMA-queue waits into dedicated copies so the
        # wide compute instructions (1-2 HW wait slots) only wait on engine
        # semaphores.
        tchV = consts.tile([1, 1], F32, name="tchV", tag="tchV")
        tchA = consts.tile([1, 1], F32, name="tchA", tag="tchA")

        def dve_touch(ap):
            nc.vector.tensor_copy(tchV[:], ap[0:1, 0:1])

        def act_touch(ap):
            nc.scalar.copy(tchA[:], ap[0:1, 0:1])

        # PE touch of the identity so later transposes don't carry its wait.
        psI = psT_p.tile([1, P], BF16, name="psI", tag="psMI", bufs=1)
        nc.tensor.transpose(psI[:], identb[:, 0:1], identb[:])

        # DMA constants in. One dma_start per tile (Tile deps are per-tile, and
        # matmuls only have ~2 wait slots); big constants are split into
        # separate ref-group tiles so compute can start after the first group.
        # Small/query-side tiles go FIRST in the (single) DMA queue so the
        # first score matmuls and label prep aren't gated on the full xT load.
        GROUP = min(R, 1024)
        NG = R // GROUP
        qTt = consts.tile([P, DCH * Q], BF16, name="qTt", tag="qTt")
        nc.sync.dma_start(qTt[:], qT_ap[:])
        ylabt = consts.tile([P, RT], F32, name="ylabt", tag="ylabt")
        nc.sync.dma_start(ylabt[:], yl_ap[:])
        iot = consts.tile([P, C], F32, name="iot", tag="iot")
        nc.sync.dma_start(iot[:], io_ap[:])
        if qaux_ap is not None:
            qauxt = consts.tile([P, nqaux], F32, name="qauxt", tag="qauxt")
            nc.sync.dma_start(qauxt[:], qaux_ap[:])
        if gm:
            muTt = consts.tile([P, DCH * C], BF16, name="muTt", tag="muTt")
            nc.sync.dma_start(muTt[:], muT_ap[:])
            emut = consts.tile([P, C], F32, name="emut", tag="emut")
            nc.sync.dma_start(emut[:], emu_ap[:])
        ones2 = consts.tile([2, P], BF16, name="ones2", tag="ones2")
        nc.vector.memset(ones2[:], 1.0)
        xTs = [[None] * NG for _ in range(DCH)]
        bbts = []
        for g in range(NG):
            for d in range(DCH):
                t = consts.tile(
                    [P, GROUP], BF16, name=f"xTs{d}_{g}", tag=f"xTs{d}_{g}"
                )
                nc.sync.dma_start(
                    t[:], xT_ap[:, d * R + g * GROUP: d * R + (g + 1) * GROUP]
                )
                xTs[d][g] = t
            t = consts.tile([2, GROUP], BF16, name=f"bbt{g}", tag=f"bbt{g}")
            nc.sync.dma_start(t[:], bb_ap[:, g * GROUP:(g + 1) * GROUP])
            bbts.append(t)
        dve_touch(iot)
        dve_touch(ylabt)
        # one-hot labels built on device: yoht[:, i*C:(i+1)*C] = (iota == y_r)
        yoht = consts.tile([P, RT * C], BF16, name="yoht", tag="yoht")
        for i in range(RT):
            nc.vector.tensor_scalar(
                out=yoht[:, i * C:(i + 1) * C], in0=iot[:],
                scalar1=ylabt[:, i:i + 1], scalar2=None, op0=ALU.is_equal,
            )
        if qaux_ap is not None:
            dve_touch(qauxt)
            act_touch(qauxt)
        if gm:
            dve_touch(emut)

        R2 = R // 2
        HT = RT // 2  # mask tiles per half

        def emit_counts(b, halves):
            """Counts + mode (+ gm) for query block b given its mask halves."""
            psc = psC_p.tile([P, C], F32, name="psC", tag="psC")
            GT = min(8, RT)  # transposes batched per PSUM bank / ACT copy
            for i0 in range(0, RT, GT):
                pst = psT_p.tile([P, GT * P], BF16, name="psT", tag="psT")
                for u in range(GT):
                    i = i0 + u
                    mh = halves[i // HT]
                    lo = (i % HT) * P
                    nc.tensor.transpose(
                        pst[:, u * P:(u + 1) * P], mh[:, lo:lo + P], identb[:]
                    )
                mTg = maskp.tile([P, GT * P], BF16, name="mTg", tag="mTg", bufs=3)
                nc.scalar.copy(mTg[:], pst[:])
                for u in range(GT):
                    i = i0 + u
                    nc.tensor.matmul(
                        psc[:],
                        mTg[:, u * P:(u + 1) * P],
                        yoht[:, i * C:(i + 1) * C],
                        start=(i == 0),
                        stop=(i == RT - 1),
                    )
            counts = small.tile([P, C], F32, name="counts", tag="counts")
            if self_exclude or gm:
                yh = small.tile([P, C], F32, name="yh", tag="yh")
                nc.vector.tensor_scalar(
                    out=yh[:],
                    in0=iot[:],
                    scalar1=qauxt[:, b:b + 1],
                    scalar2=None,
                    op0=ALU.is_equal,
                )
            if self_exclude:
                nc.vector.tensor_sub(counts[:], psc[:], yh[:])
            else:
                nc.vector.tensor_copy(counts[:], psc[:])
            # mode = first argmax of counts
            maxc = small.tile([P, 1], F32, name="maxc", tag="maxc")
            nc.vector.reduce_max(maxc[:], counts[:], axis=AX.X)
            lt01 = small.tile([P, C], F32, name="lt01", tag="lt01")
            nc.vector.tensor_scalar(
                out=lt01[:], in0=counts[:], scalar1=maxc[:], scalar2=None,
                op0=ALU.is_lt,
            )
            cand = small.tile([P, C], F32, name="cand", tag="cand")
            nc.vector.scalar_tensor_tensor(
                out=cand[:], in0=lt01[:], scalar=1e9, in1=iot[:],
                op0=ALU.mult, op1=ALU.add,
            )
            ym = small.tile([P, 1], F32, name="ym", tag="ym")
            nc.vector.tensor_reduce(ym[:], cand[:], axis=AX.X, op=ALU.min)
            nc.sync.dma_start(ym_ap[b], ym[:])
            # gaussian-mixture per-row loss
            if gm:
                psg = psG_p.tile([P, C], F32, name="psG", tag="psG")
                for d in range(DCH):
                    nc.tensor.matmul(
                        psg[:],
                        qTt[:, d * Q + b * P: d * Q + (b + 1) * P],
                        muTt[:, d * C:(d + 1) * C],
                        start=(d == 0),
                        stop=(d == DCH - 1),
                    )
                eg = small.tile([P, C], F32, name="eg", tag="eg")
                nc.scalar.activation(
                    eg[:], psg[:], mybir.ActivationFunctionType.Exp,
                    bias=qauxt[:, QB + b:QB + b + 1], scale=1.0,
                )
                piu = small.tile([P, C], F32, name="piu", tag="piu")
                nc.vector.tensor_mul(piu[:], eg[:], emut[:])
                srow = small.tile([P, 1], F32, name="srow", tag="srow")
                nc.vector.reduce_sum(srow[:], piu[:], axis=AX.X)
                nc.vector.tensor_scalar_add(srow[:], srow[:], 1e-15)
                rec = small.tile([P, 1], F32, name="rec", tag="rec")
                nc.vector.reciprocal(rec[:], srow[:])
                pin = small.tile([P, C], F32, name="pin", tag="pin")
                nc.vector.tensor_scalar(
                    out=pin[:], in0=piu[:], scalar1=rec[:], scalar2=None,
                    op0=ALU.mult,
                )
                diff = small.tile([P, C], F32, name="diff", tag="diff")
                nc.vector.tensor_sub(diff[:], pin[:], yh[:])
                sq = small.tile([P, C], F32, name="sq", tag="sq")
                nc.vector.tensor_mul(sq[:], diff[:], diff[:])
                lg = small.tile([P, 1], F32, name="lg", tag="lg")
                nc.vector.reduce_sum(lg[:], sq[:], axis=AX.X)
                nc.sync.dma_start(lg_ap[b], lg[:])

        # Software pipeline: block b's counts/mode are emitted after block
        # b+1's scores/threshold/compare, so the PE's counts work overlaps the
        # DVE threshold tail of the next block.
        pending = None
        for b in range(QB):
            # ---- scores S[q, r] = q.r - bb_r/2 for this 128-query block
            # (rank-equivalent to 2*q.r - bb_r; bb folded into the matmul) ----
            S = sbig.tile([P, R], F32, name="S", tag="S")
            for j in range(RCH):
                g, go = (j * 512) // GROUP, (j * 512) % GROUP
                ps = psS_p.tile([P, 512], F32, name="psS", tag="psS")
                for d in range(DCH):
                    nc.tensor.matmul(
                        ps[:],
                        qTt[:, d * Q + b * P: d * Q + (b + 1) * P],
                        xTs[d][g][:, go:go + 512],
                        start=(d == 0),
                        stop=False,
                    )
                nc.tensor.matmul(
                    ps[:],
                    ones2[:],
                    bbts[g][:, go:go + 512],
                    start=False,
                    stop=True,
                )
                nc.scalar.copy(S[:, j * 512:(j + 1) * 512], ps[:])
            # ---- threshold t = k-th largest score of the row ----
            if _stages < 2:
                nc.vector.max(out=small.tile([P, 8], F32, name="mdum", tag="mdum"), in_=S[:, 0:512])
                pending = None
                continue
            m1 = small.tile([P, 8], F32, name="m1", tag="m1", bufs=2)
            nc.vector.max(out=m1[:], in_=S[:])
            if k <= 8:
                mt, col = m1, k - 1
            else:
                Ssc = sbig.tile([P, R], F32, name="Ssc", tag="Ssc", bufs=1)
                nc.vector.match_replace(
                    out=Ssc[:], in_to_replace=m1[:], in_values=S[:], imm_value=-1e30
                )
                m2 = small.tile([P, 8], F32, name="m2", tag="m2", bufs=2)
                nc.vector.max(out=m2[:], in_=Ssc[:])
                mt, col = m2, k - 9
            # ---- mask[q, r] = S >= t_q, in two halves for finer overlap ----
            halves = []
            for h in range(2):
                mh = maskp.tile([P, R2], BF16, name="mh", tag="mh", bufs=3)
                nc.vector.tensor_scalar(
                    out=mh[:], in0=S[:, h * R2:(h + 1) * R2],
                    scalar1=mt[:, col:col + 1], scalar2=None, op0=ALU.is_ge,
                )
                halves.append(mh)
            if _stages >= 3 and pending is not None:
                emit_counts(*pending)
            pending = (b, halves)
        if _stages >= 3:
            emit_counts(*pending)
    nc.compile()
    return nc


def build_program_b(R, Q, D, C, n_cores=NCORES):
    """Phase B: top-8 neighbor indices per row (uint16, host decodes the
    3-NN mode) + per-row gaussian-mixture loss, SPMD over cores."""
    DCH, RCH, QB = D // P, R // 512, Q // P
    assert D % P == 0 and R % 512 == 0 and Q % P == 0

    nc = bacc.Bacc(
        "TRN2", target_bir_lowering=False, debug=False, num_devices=n_cores
    )
    xT_ap = nc.dram_tensor("xT", [P, DCH * R], BF16, kind="ExternalInput").ap()
    qT_ap = nc.dram_tensor("qT", [P, DCH * Q], BF16, kind="ExternalInput").ap()
    bb_ap = nc.dram_tensor("bbhl", [2, R], BF16, kind="ExternalInput").ap()
    io_ap = nc.dram_tensor("iotaf", [P, C], F32, kind="ExternalInput").ap()
    # qaux col b = own labels of block b; col QB+b = -aa/2 (exp bias)
    qaux_ap = nc.dram_tensor("qaux", [P, 2 * QB], F32, kind="ExternalInput").ap()
    muT_ap = nc.dram_tensor("muT", [P, DCH * C], BF16, kind="ExternalInput").ap()
    emu_ap = nc.dram_tensor("emu", [P, C], F32, kind="ExternalInput").ap()
    idx_ap = nc.dram_tensor("idxo", [QB, P, 8], U16, kind="ExternalOutput").ap()
    lg_ap = nc.dram_tensor("lgm", [QB, P, 1], F32, kind="ExternalOutput").ap()

    with tile.TileContext(nc) as tc, ExitStack() as ctx:
        consts = ctx.enter_context(tc.tile_pool(name="consts", bufs=1))
        sbig = ctx.enter_context(tc.tile_pool(name="sbig", bufs=2))
        small = ctx.enter_context(tc.tile_pool(name="small", bufs=1))
        psS_p = ctx.enter_context(tc.tile_pool(name="psS", bufs=3, space="PSUM"))
        psG_p = ctx.enter_context(tc.tile_pool(name="psG", bufs=2, space="PSUM"))

        tchV = consts.tile([1, 1], F32, name="tchV", tag="tchV")
        tchA = consts.tile([1, 1], F32, name="tchA", tag="tchA")

        def dve_touch(ap):
            nc.vector.tensor_copy(tchV[:], ap[0:1, 0:1])

        def act_touch(ap):
            nc.scalar.copy(tchA[:], ap[0:1, 0:1])

        GROUP = min(R, 1024)
        NG = R // GROUP
        qTt = consts.tile([P, DCH * Q], BF16, name="qTt", tag="qTt")
        nc.sync.dma_start(qTt[:], qT_ap[:])
        iot = consts.tile([P, C], F32, name="iot", tag="iot")
        nc.sync.dma_start(iot[:], io_ap[:])
        qauxt = consts.tile([P, 2 * QB], F32, name="qauxt", tag="qauxt")
        nc.sync.dma_start(qauxt[:], qaux_ap[:])
        muTt = consts.tile([P, DCH * C], BF16, name="muTt", tag="muTt")
        nc.sync.dma_start(muTt[:], muT_ap[:])
        emut = consts.tile([P, C], F32, name="emut", tag="emut")
        nc.sync.dma_start(emut[:], emu_ap[:])
        ones2 = consts.tile([2, P], BF16, name="ones2", tag="ones2")
        nc.vector.memset(ones2[:], 1.0)
        xTs = [[None] * NG for _ in range(DCH)]
        bbts = []
        for g in range(NG):
            for d in range(DCH):
                t = consts.tile(
                    [P, GROUP], BF16, name=f"xTs{d}_{g}", tag=f"xTs{d}_{g}"
                )
                nc.sync.dma_start(
                    t[:], xT_ap[:, d * R + g * GROUP: d * R + (g + 1) * GROUP]
                )
                xTs[d][g] = t
            t = consts.tile([2, GROUP], BF16, name=f"bbt{g}", tag=f"bbt{g}")
            nc.sync.dma_start(t[:], bb_ap[:, g * GROUP:(g + 1) * GROUP])
            bbts.append(t)
        dve_touch(iot)
        dve_touch(qauxt)
        act_touch(qauxt)
        dve_touch(emut)

        R2 = R // 2
        for b in range(QB):
            # ---- scores S[q, r] = q.r - bb_r/2 for this 128-query block ----
            S = sbig.tile([P, R], F32, name="S", tag="S")
            for j in range(RCH):
                g, go = (j * 512) // GROUP, (j * 512) % GROUP
                ps = psS_p.tile([P, 512], F32, name="psS", tag="psS")
                for d in range(DCH):
                    nc.tensor.matmul(
                        ps[:],
                        qTt[:, d * Q + b * P: d * Q + (b + 1) * P],
                        xTs[d][g][:, go:go + 512],
                        start=(d == 0),
                        stop=False,
                    )
                nc.tensor.matmul(
                    ps[:],
                    ones2[:],
                    bbts[g][:, go:go + 512],
                    start=False,
                    stop=True,
                )
                nc.scalar.copy(S[:, j * 512:(j + 1) * 512], ps[:])
            # ---- gm matmuls ride behind the scores on the PE ----
            psg = psG_p.tile([P, C], F32, name="psG", tag="psG")
            for d in range(DCH):
                nc.tensor.matmul(
                    psg[:],
                    qTt[:, d * Q + b * P: d * Q + (b + 1) * P],
                    muTt[:, d * C:(d + 1) * C],
                    start=(d == 0),
                    stop=(d == DCH - 1),
                )
            eg = small.tile([P, C], F32, name="eg", tag="eg", bufs=2)
            nc.scalar.activation(
                eg[:], psg[:], mybir.ActivationFunctionType.Exp,
                bias=qauxt[:, QB + b:QB + b + 1], scale=1.0,
            )
            # ---- top-8: two half-row max8s merged, one index scan ----
            m16 = small.tile([P, 16], F32, name="m16", tag="m16", bufs=2)
            nc.vector.max(out=m16[:, 0:8], in_=S[:, 0:R2])
            nc.vector.max(out=m16[:, 8:16], in_=S[:, R2:R])
            m8 = small.tile([P, 8], F32, name="m8", tag="m8", bufs=2)
            nc.vector.max(out=m8[:], in_=m16[:])
            idx16 = small.tile([P, 8], U16, name="idx16", tag="idx16", bufs=2)
            nc.vector.max_index(idx16[:], m8[:], S[:])
            nc.sync.dma_start(idx_ap[b], idx16[:])
            # ---- gaussian-mixture per-row loss ----
            yh = small.tile([P, C], F32, name="yh", tag="yh")
            nc.vector.tensor_scalar(
                out=yh[:], in0=iot[:], scalar1=qauxt[:, b:b + 1],
                scalar2=None, op0=ALU.is_equal,
            )
            piu = small.tile([P, C], F32, name="piu", tag="piu")
            nc.vector.tensor_mul(piu[:], eg[:], emut[:])
            srow = small.tile([P, 1], F32, name="srow", tag="srow")
            nc.vector.reduce_sum(srow[:], piu[:], axis=AX.X)
            nc.vector.tensor_scalar_add(srow[:], srow[:], 1e-15)
            rec = small.tile([P, 1], F32, name="rec", tag="rec")
            nc.vector.reciprocal(rec[:], srow[:])
            pin = small.tile([P, C], F32, name="pin", tag="pin")
            nc.vector.tensor_scalar(
                out=pin[:], in0=piu[:], scalar1=rec[:], scalar2=None,
                op0=ALU.mult,
            )
            diff = small.tile([P, C], F32, name="diff", tag="diff")
            nc.vector.tensor_sub(diff[:], pin[:], yh[:])
            sq = small.tile([P, C], F32, name="sq", tag="sq")
            nc.vector.tensor_mul(sq[:], diff[:], diff[:])
            lg = small.tile([P, 1], F32, name="lg", tag="lg")
            nc.vector.reduce_sum(lg[:], sq[:], axis=AX.X)
            nc.sync.dma_start(lg_ap[b], lg[:])
    nc.compile()
    return nc


# ---------------- host-side packing helpers ----------------

def pack_T(m):
    """[R, D] fp32 -> bf16 [P, (D//P)*R]: column block d holds rows d*P..(d+1)*P
    of m.T (i.e. element (p, d*R + r) = m[r, d*P + p])."""
    R, D = m.shape
    DCH = D // P
    mt = np.ascontiguousarray(m.T.astype(BF16_NP))  # [D, R]
    return np.ascontiguousarray(
        mt.reshape(DCH, P, R).transpose(1, 0, 2).reshape(P, DCH * R)
    )


def pack_bbhl(bb):
    """[R] fp32 -> [2, R] bf16 hi/lo split of -bb/2 (exact to ~2^-17 rel)."""
    t = (-0.5 * bb).astype(np.float32)
    hi = t.astype(BF16_NP)
    lo = (t - hi.astype(np.float32)).astype(BF16_NP)
    return np.ascontiguousarray(np.stack([hi, lo]))


def pack_cols(v):
    """[Q] -> [P, Q//P] fp32: column b = v[b*P:(b+1)*P]."""
    QB = v.shape[0] // P
    return np.ascontiguousarray(v.reshape(QB, P).T.astype(np.float32))


def mode_rows_host(vals):
    """[M, K] labels -> [M] torch.mode semantics (most frequent, smallest on
    ties)."""
    eq = vals[:, :, None] == vals[:, None, :]
    counts = eq.sum(axis=2)
    maxc = counts.max(axis=1, keepdims=True)
    masked = np.where(counts == maxc, vals, np.inf)
    return masked.min(axis=1)


_PROGRAMS = {}
LAST_EXEC_NS = None
_EXEC_NS = {}


def _get_program(key, builder):
    if key not in _PROGRAMS:
        _PROGRAMS[key] = builder()
    return _PROGRAMS[key]


def _run(nc, in_maps, phase):
    import os

    kwargs = {}
    if os.environ.get("KERNEL_TRACE"):
        kwargs = dict(trace=True, trace_cores=[0])
    t0 = _time.time()
    res = run_bass_kernel_spmd(
        nc, in_maps, core_ids=list(range(NCORES)), **kwargs
    )
    if os.environ.get("KERNEL_TIME"):
        print(f"phase {phase} dispatch+exec: {_time.time() - t0:.3f}s")
    if res.exec_time_ns:
        _EXEC_NS[phase] = res.exec_time_ns
        if res.instructions_and_trace:
            print(f"phase {phase}: {res.exec_time_ns} ns, "
                  f"trace: {res.instructions_and_trace[1]}")
    global LAST_EXEC_NS
    if len(_EXEC_NS) == 2:
        LAST_EXEC_NS = sum(_EXEC_NS.values())
    return res


def kernel(x, y, lam, perm):
    x = np.asarray(x, dtype=np.float32)
    y = np.asarray(y, dtype=np.float32)
    lam = np.float32(np.asarray(lam))
    perm = np.asarray(perm, dtype=np.int32)
    N, D = x.shape
    C = CLASSES
    x_ul = (x * lam + x[perm] * (np.float32(1.0) - lam)).astype(np.float32)

    iota_in = np.ascontiguousarray(
        np.broadcast_to(np.arange(C, dtype=np.float32), (P, C))
    )

    # ---------------- phase 1: pseudo-labels via 11-NN mode ----------------
    QA = N // NCORES
    ncA = _get_program(
        ("A", N, QA, D), lambda: build_program(N, QA, D, C, 11, False, False)
    )
    xT_in = pack_T(x)
    bb_x = (x.astype(np.float64) ** 2).sum(1).astype(np.float32)
    bb_in = pack_bbhl(bb_x)
    ylab_in = pack_cols(y)
    in_maps = []
    for c in range(NCORES):
        in_maps.append(
            {
                "xT": xT_in,
                "qT": pack_T(x_ul[c * QA:(c + 1) * QA]),
                "bbhl": bb_in,
                "ylab": ylab_in,
                "iotaf": iota_in,
            }
        )
    # Phase B's big packings depend only on x/x_ul (not on phase A's labels):
    # overlap them with phase A's transfer+execution in a background thread.
    import threading

    xc = np.concatenate([x, x_ul], axis=0)
    _bg = {}

    def _pack_b():
        _bg["xcT"] = pack_T(xc)
        _bg["qTs"] = [
            pack_T(xc[c * (2 * N) // NCORES:(c + 1) * (2 * N) // NCORES])
            for c in range(NCORES)
        ]
        aa_ = (xc.astype(np.float64) ** 2).sum(1).astype(np.float32)
        _bg["aa"] = aa_
        _bg["bbhl2"] = pack_bbhl(aa_)

    _th = threading.Thread(target=_pack_b)
    _th.start()
    resA = _run(ncA, in_maps, "A")
    _th.join()
    y_ul = np.concatenate(
        [r["ymode"].reshape(QA) for r in resA.results]
    ).astype(np.float32)

    # ---------------- host glue: per-class means ----------------
    yc = np.concatenate([y, y_ul], axis=0)
    num = xc.shape[0]
    yi = yc.astype(np.int32)
    counts = np.bincount(yi, minlength=C).astype(np.float32)
    mu = np.zeros((C, D), dtype=np.float32)
    np.add.at(mu, yi, xc)
    mu = mu / np.maximum(counts, 1.0)[:, None]
    bbm = (mu.astype(np.float64) ** 2).sum(1)
    emu = (np.exp(-bbm / 2.0) * (counts > 0)).astype(np.float32)
    emu_in = np.ascontiguousarray(np.broadcast_to(emu, (P, C)))
    aa = _bg["aa"]
    bb_in2 = _bg["bbhl2"]
    muT_in = pack_T(mu)
    xcT_in = _bg["xcT"]

    # ---------------- phase 2: 3-NN indices + gm loss rows ----------------
    QB_ = num // NCORES
    ncB = _get_program(
        ("B", num, QB_, D), lambda: build_program_b(num, QB_, D, C)
    )
    in_maps = []
    for c in range(NCORES):
        sl = slice(c * QB_, (c + 1) * QB_)
        qaux = np.concatenate(
            [pack_cols(yc[sl]), pack_cols(-0.5 * aa[sl])], axis=1
        ).astype(np.float32)
        in_maps.append(
            {
                "xT": xcT_in,
                "qT": _bg["qTs"][c],
                "bbhl": bb_in2,
                "iotaf": iota_in,
                "qaux": np.ascontiguousarray(qaux),
                "muT": muT_in,
                "emu": emu_in,
            }
        )
    resB = _run(ncB, in_maps, "B")
    # idxo[b, p, j] = j-th nearest ref of query (b*128 + p); rank 0 is self.
    idx_all = np.concatenate(
        [r["idxo"].reshape(QB_, 8) for r in resB.results]
    ).astype(np.int64)
    lgm_rows = np.concatenate([r["lgm"].reshape(QB_) for r in resB.results])

    y_ng = mode_rows_host(yc[idx_all[:, 1:4]]).astype(np.float32)

    loss_gm = np.float32(lgm_rows.mean(dtype=np.float64))
    loss_knn = np.float32(((y_ng - yc) ** 2).mean(dtype=np.float64))
    return np.float32(loss_gm + np.float32(0.01) * loss_knn)


# revision 15
# speedup vs baseline: 1.4060x; 1.1057x over previous
"""Trainium2 Bass kernel for nn_DGMMLoss (retrieval_knn).

Reference computation (see problem statement):
  1. x_ul = lam*x + (1-lam)*x[perm]; pseudo-label via mode of 11-NN labels
  2. concat; per-class means; gaussian-mixture loss term
  3. kNN regularizer: mode of 3-NN (self-excluded) labels, MSE
  loss = loss_gm + 0.01 * loss_knn

Device strategy (8 NeuronCores, data-parallel over query rows; two SPMD
launches):

Launch K (one program, ~all the FLOPs): both kNN problems share the ref set
  xc = [x; x_ul] (phase A only scans the x half), so one 8MB bf16 xcT load
  feeds both. Scores s[q,r] = q.r - ||r||^2/2 via bf16 matmuls (fp32 psum);
  the -bb/2 term rides in the same accumulation as an augmented K=2
  contraction of a ones column against a bf16 hi/lo split (exact to ~2^-17
  rel), so psum evacuation is a pure ACT copy.
  - A-part (11-NN pseudo-labels, 4 query blocks/core): per-row 11th-largest
    via DVE max8+match_replace+max8; tensor_scalar is_ge gives a bf16
    mask[q,r]; per-class counts = maskT.T @ onehot(y) on the PE (mask tiles
    transposed on the PE, batched per PSUM bank, evacuated by ACT); mode =
    first argmax (smallest class on ties, = torch.mode) on DVE.
  - B-part (3-NN indices, 8 query blocks/core): top-8 values from two
    half-row max8s merged by a 16-wide max8; one DVE max_index scan yields
    uint16 column indices, DMA'd to the host. Self is always rank 0 (score
    gap орders above bf16 noise), so the host mode over label ranks 1..3
    reproduces the self-excluded 3-NN mode. No transposes/counts matmuls.
  A blocks are emitted first (their DVE work covers the xcT DMA tail), then
  B blocks, software-pipelined so the DVE never head-blocks.

Launch G (tiny): gaussian-mixture rows. Needs per-class means, which the
  host computes from phase A's pseudo-labels. Per 128-query block: 4 PE
  matmuls q.muT (100 cols), ACT exp(. - aa/2), and a short DVE chain
  (normalize, subtract onehot, fused square+reduce) -> per-row loss.

Host does only O(N*D) glue: x_ul, norms, packing, per-class means,
label-gather + mode-of-3 from device indices, final scalar assembly. bf16
scoring shifts the loss by ~1e-3 relative (verified against an fp64 model;
fp32 matmul on TRN2 is 4x slower than bf16).
"""

from contextlib import ExitStack

import numpy as np
import ml_dtypes

import time as _time

import concourse.bacc as bacc
import concourse.tile as tile
import concourse.mybir as mybir
from concourse.bass_utils import run_bass_kernel_spmd
from concourse.masks import make_identity

P = 128
NCORES = 8
CLASSES = 100
F32 = mybir.dt.float32
BF16 = mybir.dt.bfloat16
U16 = mybir.dt.uint16
BF16_NP = ml_dtypes.bfloat16
ALU = mybir.AluOpType
AX = mybir.AxisListType


def build_knn(R, RA, QA, QB, D, C, kA, n_cores=NCORES):
    """Merged kNN launch: A-part = 11-NN mode over the first RA refs for QA
    queries; B-part = top-8 neighbor indices over all R refs for QB queries.
    """
    DCH = D // P
    RTA, RCHA, QAB = RA // P, RA // 512, QA // P
    RCHB, QBB = R // 512, QB // P
    assert D % P == 0 and R % 1024 == 0 and RA % 1024 == 0 and 8 < kA <= 16

    nc = bacc.Bacc(
        "TRN2", target_bir_lowering=False, debug=False, num_devices=n_cores
    )
    xT_ap = nc.dram_tensor("xcT", [P, DCH, R], BF16, kind="ExternalInput").ap()
    qa_ap = nc.dram_tensor("qTa", [P, DCH * QA], BF16, kind="ExternalInput").ap()
    qb_ap = nc.dram_tensor("qTb", [P, DCH * QB], BF16, kind="ExternalInput").ap()
    bb_ap = nc.dram_tensor("bbhl", [2, R], BF16, kind="ExternalInput").ap()
    yl_ap = nc.dram_tensor("ylab", [P, RTA], F32, kind="ExternalInput").ap()
    io_ap = nc.dram_tensor("iotaf", [P, C], F32, kind="ExternalInput").ap()
    ym_ap = nc.dram_tensor("ymode", [QAB, P, 1], F32, kind="ExternalOutput").ap()
    idx_ap = nc.dram_tensor("idxo", [QBB, P, 8], U16, kind="ExternalOutput").ap()

    with tile.TileContext(nc) as tc, ExitStack() as ctx:
        consts = ctx.enter_context(tc.tile_pool(name="consts", bufs=1))
        sbig = ctx.enter_context(tc.tile_pool(name="sbig", bufs=2))
        maskp = ctx.enter_context(tc.tile_pool(name="maskp", bufs=1))
        small = ctx.enter_context(tc.tile_pool(name="small", bufs=1))
        psS_p = ctx.enter_context(tc.tile_pool(name="psS", bufs=3, space="PSUM"))
        psT_p = ctx.enter_context(tc.tile_pool(name="psT", bufs=2, space="PSUM"))
        psC_p = ctx.enter_context(tc.tile_pool(name="psC", bufs=1, space="PSUM"))

        identb = consts.tile([P, P], BF16, name="identb", tag="identb")
        make_identity(nc, identb)

        # Tiny "touch" ops absorb DMA-queue waits into dedicated copies so the
        # wide compute instructions (1-2 HW wait slots) only wait on engine
        # semaphores.
        tchV = consts.tile([1, 1], F32, name="tchV", tag="tchV")

        def dve_touch(ap):
            nc.vector.tensor_copy(tchV[:], ap[0:1, 0:1])

        # PE touch of the identity so later transposes don't carry its wait.
        psI = psT_p.tile([1, P], BF16, name="psI", tag="psMI", bufs=1)
        nc.tensor.transpose(psI[:], identb[:, 0:1], identb[:])

        # DMA constants in; small/label-side tiles first so the yoht build and
        # the A-part aren't gated on the full xcT load; qTb (B-part only)
        # after the A-part ref groups. Each ref group is ONE strided DMA of
        # all DCH d-slices (DMA issue costs ~650ns each; fewer is faster).
        GROUP = 1024
        NG = R // GROUP
        NGA = RA // GROUP
        ylabt = consts.tile([P, RTA], F32, name="ylabt", tag="ylabt")
        nc.sync.dma_start(ylabt[:], yl_ap[:])
        iot = consts.tile([P, C], F32, name="iot", tag="iot")
        nc.sync.dma_start(iot[:], io_ap[:])
        qTa = consts.tile([P, DCH * QA], BF16, name="qTa", tag="qTa")
        nc.sync.dma_start(qTa[:], qa_ap[:])
        bbt = consts.tile([2, R], BF16, name="bbt", tag="bbt")
        nc.sync.dma_start(bbt[:], bb_ap[:])
        ones2 = consts.tile([2, P], BF16, name="ones2", tag="ones2")
        nc.vector.memset(ones2[:], 1.0)
        xgs = [None] * NG
        qTb = consts.tile([P, DCH * QB], BF16, name="qTb", tag="qTb")

        def load_group(g):
            t = consts.tile([P, DCH, GROUP], BF16, name=f"xg{g}", tag=f"xg{g}")
            nc.sync.dma_start(t[:], xT_ap[:, :, g * GROUP:(g + 1) * GROUP])
            xgs[g] = t

        for g in range(NGA):
            load_group(g)
        nc.sync.dma_start(qTb[:], qb_ap[:])
        for g in range(NGA, NG):
            load_group(g)
        dve_touch(iot)
        dve_touch(ylabt)
        # one-hot labels built on device: yoht[:, i*C:(i+1)*C] = (iota == y_r)
        yoht = consts.tile([P, RTA * C], BF16, name="yoht", tag="yoht")
        for i in range(RTA):
            nc.vector.tensor_scalar(
                out=yoht[:, i * C:(i + 1) * C], in0=iot[:],
                scalar1=ylabt[:, i:i + 1], scalar2=None, op0=ALU.is_equal,
            )

        def scores(b, qt, Qtot, rch):
            """S[q, r] = q.r - bb_r/2 for query block b (queries from qt)."""
            S = sbig.tile([P, R], F32, name="S", tag="S")
            for j in range(rch):
                g, go = (j * 512) // GROUP, (j * 512) % GROUP
                ps = psS_p.tile([P, 512], F32, name="psS", tag="psS")
                for d in range(DCH):
                    nc.tensor.matmul(
                        ps[:],
                        qt[:, d * Qtot + b * P: d * Qtot + (b + 1) * P],
                        xgs[g][:, d, go:go + 512],
                        start=(d == 0),
                        stop=False,
                    )
                nc.tensor.matmul(
                    ps[:],
                    ones2[:],
                    bbt[:, j * 512:(j + 1) * 512],
                    start=False,
                    stop=True,
                )
                nc.scalar.copy(S[:, j * 512:(j + 1) * 512], ps[:])
            return S

        RA2 = RA // 2
        HTA = RTA // 2  # A-part mask tiles per half

        def a_thresh(b, S):
            """11th-largest threshold + bf16 mask halves for A block b."""
            m1 = small.tile([P, 8], F32, name="m1", tag="m1", bufs=2)
            nc.vector.max(out=m1[:], in_=S[:, 0:RA])
            Ssc = sbig.tile([P, RA], F32, name="Ssc", tag="Ssc", bufs=1)
            nc.vector.match_replace(
                out=Ssc[:], in_to_replace=m1[:], in_values=S[:, 0:RA],
                imm_value=-1e30,
            )
            m2 = small.tile([P, 8], F32, name="m2", tag="m2", bufs=2)
            nc.vector.max(out=m2[:], in_=Ssc[:])
            halves = []
            for h in range(2):
                mh = maskp.tile([P, RA2], BF16, name="mh", tag="mh", bufs=3)
                nc.vector.tensor_scalar(
                    out=mh[:], in0=S[:, h * RA2:(h + 1) * RA2],
                    scalar1=m2[:, kA - 9:kA - 8], scalar2=None, op0=ALU.is_ge,
                )
                halves.append(mh)
            return halves

        def a_counts(b, halves):
            """Counts + mode for A query block b given its mask halves."""
            psc = psC_p.tile([P, C], F32, name="psC", tag="psC")
            GT = 8  # transposes batched per PSUM bank / ACT copy
            for i0 in range(0, RTA, GT):
                pst = psT_p.tile([P, GT * P], BF16, name="psT", tag="psT")
                for u in range(GT):
                    i = i0 + u
                    mh = halves[i // HTA]
                    lo = (i % HTA) * P
                    nc.tensor.transpose(
                        pst[:, u * P:(u + 1) * P], mh[:, lo:lo + P], identb[:]
                    )
                mTg = maskp.tile([P, GT * P], BF16, name="mTg", tag="mTg", bufs=3)
                nc.scalar.copy(mTg[:], pst[:])
                for u in range(GT):
                    i = i0 + u
                    nc.tensor.matmul(
                        psc[:],
                        mTg[:, u * P:(u + 1) * P],
                        yoht[:, i * C:(i + 1) * C],
                        start=(i == 0),
                        stop=(i == RTA - 1),
                    )
            counts = small.tile([P, C], F32, name="counts", tag="counts")
            nc.vector.tensor_copy(counts[:], psc[:])
            # mode = first argmax of counts
            maxc = small.tile([P, 1], F32, name="maxc", tag="maxc")
            nc.vector.reduce_max(maxc[:], counts[:], axis=AX.X)
            lt01 = small.tile([P, C], F32, name="lt01", tag="lt01")
            nc.vector.tensor_scalar(
                out=lt01[:], in0=counts[:], scalar1=maxc[:], scalar2=None,
                op0=ALU.is_lt,
            )
            cand = small.tile([P, C], F32, name="cand", tag="cand")
            nc.vector.scalar_tensor_tensor(
                out=cand[:], in0=lt01[:], scalar=1e9, in1=iot[:],
                op0=ALU.mult, op1=ALU.add,
            )
            ym = small.tile([P, 1], F32, name="ym", tag="ym")
            nc.vector.tensor_reduce(ym[:], cand[:], axis=AX.X, op=ALU.min)
            nc.sync.dma_start(ym_ap[b], ym[:])

        R2 = R // 2

        def b_max(b, S):
            """Top-8 values (two half scans + merge) and their indices."""
            m16 = small.tile([P, 16], F32, name="m16", tag="m16", bufs=2)
            nc.vector.max(out=m16[:, 0:8], in_=S[:, 0:R2])
            nc.vector.max(out=m16[:, 8:16], in_=S[:, R2:R])
            m8 = small.tile([P, 8], F32, name="m8", tag="m8", bufs=2)
            nc.vector.max(out=m8[:], in_=m16[:])
            idx16 = small.tile([P, 8], U16, name="idx16", tag="idx16", bufs=2)
            nc.vector.max_index(idx16[:], m8[:], S[:])
            nc.sync.dma_start(idx_ap[b], idx16[:])

        # A blocks first (their DVE work covers the xcT DMA tail), then B
        # blocks; counts for A block b are emitted after block b+1's
        # threshold so PE counts work overlaps the DVE threshold tail.
        pending = None
        for b in range(QAB):
            S = scores(b, qTa, QA, RCHA)
            halves = a_thresh(b, S)
            if pending is not None:
                a_counts(*pending)
            pending = (b, halves)
        Sb0 = scores(0, qTb, QB, RCHB)
        a_counts(*pending)
        for b in range(QBB):
            Sn = scores(b + 1, qTb, QB, RCHB) if b + 1 < QBB else None
            b_max(b, Sb0)
            Sb0 = Sn
    nc.compile()
    return nc


def build_gm(Q, D, C, n_cores=NCORES):
    """GM launch: per-row gaussian-mixture loss against per-class means.

    Small enough to be latency-bound, so the post-exp arithmetic is batched
    across all QBB query blocks as wide [P, QBB*C] DVE ops; only the ops
    that need a per-(block, partition) scalar (exp bias, onehot, normalize,
    square+reduce) stay per-block.
    """
    DCH, QBB = D // P, Q // P
    nc = bacc.Bacc(
        "TRN2", target_bir_lowering=False, debug=False, num_devices=n_cores
    )
    qT_ap = nc.dram_tensor("qT", [P, DCH * Q], BF16, kind="ExternalInput").ap()
    muT_ap = nc.dram_tensor("muT", [P, DCH * C], BF16, kind="ExternalInput").ap()
    # emu replicated across blocks: [P, QBB*C]
    emu_ap = nc.dram_tensor("emu", [P, QBB * C], F32, kind="ExternalInput").ap()
    # qaux col b = own labels of block b; col QBB+b = -aa/2 (exp bias)
    qaux_ap = nc.dram_tensor("qaux", [P, 2 * QBB], F32, kind="ExternalInput").ap()
    io_ap = nc.dram_tensor("iotaf", [P, C], F32, kind="ExternalInput").ap()
    lg_ap = nc.dram_tensor("lgm", [P, QBB], F32, kind="ExternalOutput").ap()

    with tile.TileContext(nc) as tc, ExitStack() as ctx:
        consts = ctx.enter_context(tc.tile_pool(name="consts", bufs=1))
        small = ctx.enter_context(tc.tile_pool(name="small", bufs=1))
        psG_p = ctx.enter_context(tc.tile_pool(name="psG", bufs=2, space="PSUM"))

        tchV = consts.tile([1, 1], F32, name="tchV", tag="tchV")
        tchA = consts.tile([1, 1], F32, name="tchA", tag="tchA")
        muTt = consts.tile([P, DCH * C], BF16, name="muTt", tag="muTt")
        nc.sync.dma_start(muTt[:], muT_ap[:])
        qTt = consts.tile([P, DCH * Q], BF16, name="qTt", tag="qTt")
        nc.sync.dma_start(qTt[:], qT_ap[:])
        emut = consts.tile([P, QBB, C], F32, name="emut", tag="emut")
        nc.sync.dma_start(emut[:], emu_ap[:])
        qauxt = consts.tile([P, 2 * QBB], F32, name="qauxt", tag="qauxt")
        nc.sync.dma_start(qauxt[:], qaux_ap[:])
        iot = consts.tile([P, C], F32, name="iot", tag="iot")
        nc.sync.dma_start(iot[:], io_ap[:])
        nc.vector.tensor_copy(tchV[:], emut[0:1, 0:1, 0:1])
        nc.vector.tensor_copy(tchV[:], qauxt[0:1, 0:1])
        nc.vector.tensor_copy(tchV[:], iot[0:1, 0:1])
        nc.scalar.copy(tchA[:], qauxt[0:1, 0:1])

        eg_all = small.tile([P, QBB, C], F32, name="eg_all", tag="eg_all")
        yh_all = small.tile([P, QBB, C], F32, name="yh_all", tag="yh_all")
        for b in range(QBB):
            psg = psG_p.tile([P, C], F32, name="psG", tag="psG")
            for d in range(DCH):
                nc.tensor.matmul(
                    psg[:],
                    qTt[:, d * Q + b * P: d * Q + (b + 1) * P],
                    muTt[:, d * C:(d + 1) * C],
                    start=(d == 0),
                    stop=(d == DCH - 1),
                )
            nc.scalar.activation(
                eg_all[:, b, :], psg[:], mybir.ActivationFunctionType.Exp,
                bias=qauxt[:, QBB + b:QBB + b + 1], scale=1.0,
            )
            nc.vector.tensor_scalar(
                out=yh_all[:, b, :], in0=iot[:], scalar1=qauxt[:, b:b + 1],
                scalar2=None, op0=ALU.is_equal,
            )
        piu_all = small.tile([P, QBB, C], F32, name="piu_all", tag="piu_all")
        nc.vector.tensor_mul(piu_all[:], eg_all[:], emut[:])
        srow8 = small.tile([P, QBB], F32, name="srow8", tag="srow8")
        nc.vector.reduce_sum(srow8[:], piu_all[:], axis=AX.X)
        nc.vector.tensor_scalar_add(srow8[:], srow8[:], 1e-15)
        rec8 = small.tile([P, QBB], F32, name="rec8", tag="rec8")
        nc.vector.reciprocal(rec8[:], srow8[:])
        lg8 = small.tile([P, QBB], F32, name="lg8", tag="lg8")
        for b in range(QBB):
            diff = small.tile([P, C], F32, name="diff", tag="diff", bufs=2)
            nc.vector.scalar_tensor_tensor(
                out=diff[:], in0=piu_all[:, b, :], scalar=rec8[:, b:b + 1],
                in1=yh_all[:, b, :], op0=ALU.mult, op1=ALU.subtract,
            )
            sqj = small.tile([P, C], F32, name="sqj", tag="sqj", bufs=2)
            nc.vector.tensor_mul(sqj[:], diff[:], diff[:])
            nc.vector.reduce_sum(lg8[:, b:b + 1], sqj[:], axis=AX.X)
        nc.sync.dma_start(lg_ap[:], lg8[:])
    nc.compile()
    return nc


# ---------------- host-side packing helpers ----------------

def pack_T(m):
    """[R, D] fp32 -> bf16 [P, (D//P)*R]: column block d holds rows d*P..(d+1)*P
    of m.T (i.e. element (p, d*R + r) = m[r, d*P + p])."""
    R, D = m.shape
    DCH = D // P
    mt = np.ascontiguousarray(m.T.astype(BF16_NP))  # [D, R]
    return np.ascontiguousarray(
        mt.reshape(DCH, P, R).transpose(1, 0, 2).reshape(P, DCH * R)
    )


def pack_bbhl(bb):
    """[R] fp32 -> [2, R] bf16 hi/lo split of -bb/2 (exact to ~2^-17 rel)."""
    t = (-0.5 * bb).astype(np.float32)
    hi = t.astype(BF16_NP)
    lo = (t - hi.astype(np.float32)).astype(BF16_NP)
    return np.ascontiguousarray(np.stack([hi, lo]))


def pack_cols(v):
    """[Q] -> [P, Q//P] fp32: column b = v[b*P:(b+1)*P]."""
    QB = v.shape[0] // P
    return np.ascontiguousarray(v.reshape(QB, P).T.astype(np.float32))


def mode_rows_host(vals):
    """[M, K] labels -> [M] torch.mode semantics (most frequent, smallest on
    ties)."""
    eq = vals[:, :, None] == vals[:, None, :]
    counts = eq.sum(axis=2)
    maxc = counts.max(axis=1, keepdims=True)
    masked = np.where(counts == maxc, vals, np.inf)
    return masked.min(axis=1)


_PROGRAMS = {}
LAST_EXEC_NS = None
_EXEC_NS = {}


def _get_program(key, builder):
    if key not in _PROGRAMS:
        _PROGRAMS[key] = builder()
    return _PROGRAMS[key]


def _run(nc, in_maps, phase):
    import os

    kwargs = {}
    if os.environ.get("KERNEL_TRACE"):
        kwargs = dict(trace=True, trace_cores=[0])
    t0 = _time.time()
    res = run_bass_kernel_spmd(
        nc, in_maps, core_ids=list(range(NCORES)), **kwargs
    )
    if os.environ.get("KERNEL_TIME"):
        print(f"phase {phase} dispatch+exec: {_time.time() - t0:.3f}s")
    if res.exec_time_ns:
        _EXEC_NS[phase] = res.exec_time_ns
        if res.instructions_and_trace:
            print(f"phase {phase}: {res.exec_time_ns} ns, "
                  f"trace: {res.instructions_and_trace[1]}")
    global LAST_EXEC_NS
    if len(_EXEC_NS) == 2:
        LAST_EXEC_NS = sum(_EXEC_NS.values())
    return res


def kernel(x, y, lam, perm):
    x = np.asarray(x, dtype=np.float32)
    y = np.asarray(y, dtype=np.float32)
    lam = np.float32(np.asarray(lam))
    perm = np.asarray(perm, dtype=np.int32)
    N, D = x.shape
    C = CLASSES
    x_ul = (x * lam + x[perm] * (np.float32(1.0) - lam)).astype(np.float32)
    xc = np.concatenate([x, x_ul], axis=0)
    num = xc.shape[0]

    iota_in = np.ascontiguousarray(
        np.broadcast_to(np.arange(C, dtype=np.float32), (P, C))
    )

    # ---------------- launch K: both kNN problems ----------------
    QA = N // NCORES
    QB_ = num // NCORES
    ncK = _get_program(
        ("K", num, N, QA, QB_, D),
        lambda: build_knn(num, N, QA, QB_, D, C, 11),
    )
    aa = (xc.astype(np.float64) ** 2).sum(1).astype(np.float32)
    xcT_in = pack_T(xc).reshape(P, D // P, num)
    bb_in = pack_bbhl(aa)
    ylab_in = pack_cols(y)
    in_maps = []
    for c in range(NCORES):
        in_maps.append(
            {
                "xcT": xcT_in,
                "qTa": pack_T(x_ul[c * QA:(c + 1) * QA]),
                "qTb": pack_T(xc[c * QB_:(c + 1) * QB_]),
                "bbhl": bb_in,
                "ylab": ylab_in,
                "iotaf": iota_in,
            }
        )
    resK = _run(ncK, in_maps, "K")
    y_ul = np.concatenate(
        [r["ymode"].reshape(QA) for r in resK.results]
    ).astype(np.float32)
    # idxo[b, p, j] = j-th nearest ref of query (b*128 + p); rank 0 is self.
    idx_all = np.concatenate(
        [r["idxo"].reshape(QB_, 8) for r in resK.results]
    ).astype(np.int64)

    # ---------------- host glue: per-class means, 3-NN mode ----------------
    yc = np.concatenate([y, y_ul], axis=0)
    yi = yc.astype(np.int32)
    counts = np.bincount(yi, minlength=C).astype(np.float32)
    mu = np.zeros((C, D), dtype=np.float32)
    np.add.at(mu, yi, xc)
    mu = mu / np.maximum(counts, 1.0)[:, None]
    bbm = (mu.astype(np.float64) ** 2).sum(1)
    emu = (np.exp(-bbm / 2.0) * (counts > 0)).astype(np.float32)
    QBB = (num // NCORES) // P
    emu_in = np.ascontiguousarray(
        np.broadcast_to(np.tile(emu, QBB), (P, QBB * C))
    )
    muT_in = pack_T(mu)
    y_ng = mode_rows_host(yc[idx_all[:, 1:4]]).astype(np.float32)

    # ---------------- launch G: gm loss rows ----------------
    ncG = _get_program(("G", QB_, D), lambda: build_gm(QB_, D, C))
    in_maps = []
    for c in range(NCORES):
        sl = slice(c * QB_, (c + 1) * QB_)
        qaux = np.concatenate(
            [pack_cols(yc[sl]), pack_cols(-0.5 * aa[sl])], axis=1
        ).astype(np.float32)
        in_maps.append(
            {
                "qT": pack_T(xc[c * QB_:(c + 1) * QB_]),
                "muT": muT_in,
                "emu": emu_in,
                "qaux": np.ascontiguousarray(qaux),
                "iotaf": iota_in,
            }
        )
    resG = _run(ncG, in_maps, "G")
    # lgm[p, b] = per-row loss of query (b*128 + p) on that core
    lgm_rows = np.concatenate(
        [r["lgm"].reshape(P, QB_ // P).T.reshape(QB_) for r in resG.results]
    )

    loss_gm = np.float32(lgm_rows.mean(dtype=np.float64))
    loss_knn = np.float32(((y_ng - yc) ** 2).mean(dtype=np.float64))
    return np.float32(loss_gm + np.float32(0.01) * loss_knn)


# revision 24
# speedup vs baseline: 1.4509x; 1.0320x over previous
"""Trainium2 Bass kernel for nn_DGMMLoss (retrieval_knn).

Reference computation (see problem statement):
  1. x_ul = lam*x + (1-lam)*x[perm]; pseudo-label via mode of 11-NN labels
  2. concat; per-class means; gaussian-mixture loss term
  3. kNN regularizer: mode of 3-NN (self-excluded) labels, MSE
  loss = loss_gm + 0.01 * loss_knn

Device strategy (8 NeuronCores, data-parallel over query rows; two SPMD
launches):

Launch K (one program, ~all the FLOPs): both kNN problems share the ref set
  xc = [x; x_ul] (phase A only scans the x half), so one 8MB bf16 xcT load
  feeds both. Scores s[q,r] = q.r - ||r||^2/2 via bf16 matmuls (fp32 psum);
  the -bb/2 term rides in the same accumulation as an augmented K=2
  contraction of a ones column against a bf16 hi/lo split (exact to ~2^-17
  rel), so psum evacuation is a pure ACT copy.
  - A-part (11-NN pseudo-labels, 4 query blocks/core): per-row 11th-largest
    via DVE max8+match_replace+max8; tensor_scalar is_ge gives a bf16
    mask[q,r]; per-class counts = maskT.T @ onehot(y) on the PE (mask tiles
    transposed on the PE, batched per PSUM bank, evacuated by ACT); mode =
    first argmax (smallest class on ties, = torch.mode) on DVE.
  - B-part (3-NN indices, 8 query blocks/core): top-8 values from two
    half-row max8s merged by a 16-wide max8; one DVE max_index scan yields
    uint16 column indices, DMA'd to the host. Self is always rank 0 (score
    gap орders above bf16 noise), so the host mode over label ranks 1..3
    reproduces the self-excluded 3-NN mode. No transposes/counts matmuls.
  A blocks are emitted first (their DVE work covers the xcT DMA tail), then
  B blocks, software-pipelined so the DVE never head-blocks.

Launch G (tiny): gaussian-mixture rows. Needs per-class means, which the
  host computes from phase A's pseudo-labels. Per 128-query block: 4 PE
  matmuls q.muT (100 cols), ACT exp(. - aa/2), and a short DVE chain
  (normalize, subtract onehot, fused square+reduce) -> per-row loss.

Host does only O(N*D) glue: x_ul, norms, packing, per-class means,
label-gather + mode-of-3 from device indices, final scalar assembly. bf16
scoring shifts the loss by ~1e-3 relative (verified against an fp64 model;
fp32 matmul on TRN2 is 4x slower than bf16).
"""

from contextlib import ExitStack

import numpy as np
import ml_dtypes

import time as _time

import concourse.bacc as bacc
import concourse.tile as tile
import concourse.mybir as mybir
from concourse.bass_utils import run_bass_kernel_spmd
from concourse.masks import make_identity

P = 128
NCORES = 8
CLASSES = 100
F32 = mybir.dt.float32
BF16 = mybir.dt.bfloat16
U16 = mybir.dt.uint16
BF16_NP = ml_dtypes.bfloat16
ALU = mybir.AluOpType
AX = mybir.AxisListType


def build_knn(R, RA, QA, QB, D, C, kA, n_cores=NCORES):
    """Merged kNN launch: A-part = 11-NN mode over the first RA refs for QA
    queries; B-part = top-8 neighbor indices over all R refs for QB queries.
    """
    DCH = D // P
    RTA, RCHA, QAB = RA // P, RA // 512, QA // P
    RCHB, QBB = R // 512, QB // P
    assert D % P == 0 and R % 1024 == 0 and RA % 1024 == 0 and 8 < kA <= 16

    nc = bacc.Bacc(
        "TRN2", target_bir_lowering=False, debug=False, num_devices=n_cores
    )
    xT_ap = nc.dram_tensor("xcT", [P, DCH, R], BF16, kind="ExternalInput").ap()
    qa_ap = nc.dram_tensor("qTa", [P, DCH * QA], BF16, kind="ExternalInput").ap()
    qb_ap = nc.dram_tensor("qTb", [P, DCH * QB], BF16, kind="ExternalInput").ap()
    bb_ap = nc.dram_tensor("bbhl", [2, R], BF16, kind="ExternalInput").ap()
    yl_ap = nc.dram_tensor("ylab", [P, RTA], F32, kind="ExternalInput").ap()
    io_ap = nc.dram_tensor("iotaf", [P, C], F32, kind="ExternalInput").ap()
    ym_ap = nc.dram_tensor("ymode", [QAB, P, 1], F32, kind="ExternalOutput").ap()
    idx_ap = nc.dram_tensor("idxo", [QBB, P, 8], U16, kind="ExternalOutput").ap()

    with tile.TileContext(nc) as tc, ExitStack() as ctx:
        consts = ctx.enter_context(tc.tile_pool(name="consts", bufs=1))
        sbig = ctx.enter_context(tc.tile_pool(name="sbig", bufs=3))
        maskp = ctx.enter_context(tc.tile_pool(name="maskp", bufs=1))
        small = ctx.enter_context(tc.tile_pool(name="small", bufs=1))
        psS_p = ctx.enter_context(tc.tile_pool(name="psS", bufs=3, space="PSUM"))
        psT_p = ctx.enter_context(tc.tile_pool(name="psT", bufs=2, space="PSUM"))
        psC_p = ctx.enter_context(tc.tile_pool(name="psC", bufs=1, space="PSUM"))

        identb = consts.tile([P, P], BF16, name="identb", tag="identb")
        make_identity(nc, identb)

        # Tiny "touch" ops absorb DMA-queue waits into dedicated copies so the
        # wide compute instructions (1-2 HW wait slots) only wait on engine
        # semaphores.
        tchV = consts.tile([1, 1], F32, name="tchV", tag="tchV")

        def dve_touch(ap):
            nc.vector.tensor_copy(tchV[:], ap[0:1, 0:1])

        # PE touch of the identity so later transposes don't carry its wait.
        psI = psT_p.tile([1, P], BF16, name="psI", tag="psMI", bufs=1)
        nc.tensor.transpose(psI[:], identb[:, 0:1], identb[:])

        # DMA constants in; small/label-side tiles first so the yoht build and
        # the A-part aren't gated on the full xcT load; qTb (B-part only)
        # after the A-part ref groups. Each ref group is ONE strided DMA of
        # all DCH d-slices (DMA issue costs ~650ns each; fewer is faster).
        GROUP = 1024
        NG = R // GROUP
        NGA = RA // GROUP
        ylabt = consts.tile([P, RTA], F32, name="ylabt", tag="ylabt")
        nc.sync.dma_start(ylabt[:], yl_ap[:])
        iot = consts.tile([P, C], F32, name="iot", tag="iot")
        nc.sync.dma_start(iot[:], io_ap[:])
        qTa = consts.tile([P, DCH * QA], BF16, name="qTa", tag="qTa")
        nc.sync.dma_start(qTa[:], qa_ap[:])
        bbt = consts.tile([2, R], BF16, name="bbt", tag="bbt")
        nc.sync.dma_start(bbt[:], bb_ap[:])
        ones2 = consts.tile([2, P], BF16, name="ones2", tag="ones2")
        nc.vector.memset(ones2[:], 1.0)
        xgs = [None] * NG
        qTb = consts.tile([P, DCH * QB], BF16, name="qTb", tag="qTb")

        def load_group(g):
            t = consts.tile([P, DCH, GROUP], BF16, name=f"xg{g}", tag=f"xg{g}")
            nc.sync.dma_start(t[:], xT_ap[:, :, g * GROUP:(g + 1) * GROUP])
            xgs[g] = t

        for g in range(NGA):
            load_group(g)
        nc.sync.dma_start(qTb[:], qb_ap[:])
        for g in range(NGA, NG):
            load_group(g)
        dve_touch(iot)
        dve_touch(ylabt)
        # one-hot labels built on device: yoht[:, i*C:(i+1)*C] = (iota == y_r).
        # Runs on the otherwise-idle Pool engine to keep the DVE free.
        yoht = consts.tile([P, RTA * C], BF16, name="yoht", tag="yoht")
        for i in range(RTA):
            nc.gpsimd.tensor_scalar(
                out=yoht[:, i * C:(i + 1) * C], in0=iot[:],
                scalar1=ylabt[:, i:i + 1], scalar2=None, op0=ALU.is_equal,
            )

        def scores(b, qt, Qtot, rch):
            """S[q, r] = q.r - bb_r/2 for query block b (queries from qt)."""
            S = sbig.tile([P, R], F32, name="S", tag="S")
            for j in range(rch):
                g, go = (j * 512) // GROUP, (j * 512) % GROUP
                ps = psS_p.tile([P, 512], F32, name="psS", tag="psS")
                for d in range(DCH):
                    nc.tensor.matmul(
                        ps[:],
                        qt[:, d * Qtot + b * P: d * Qtot + (b + 1) * P],
                        xgs[g][:, d, go:go + 512],
                        start=(d == 0),
                        stop=False,
                    )
                nc.tensor.matmul(
                    ps[:],
                    ones2[:],
                    bbt[:, j * 512:(j + 1) * 512],
                    start=False,
                    stop=True,
                )
                nc.scalar.copy(S[:, j * 512:(j + 1) * 512], ps[:])
            return S

        RA2 = RA // 2
        HTA = RTA // 2  # A-part mask tiles per half

        def a_thresh(b, S):
            """11th-largest threshold + bf16 mask halves for A block b.

            The A-part only scans refs [0, RA), so the upper half of its
            [P, R] S tile is free scratch for the match_replace output. The
            first max8 runs as two half scans + merge so block 0's first scan
            starts as soon as the first half of the refs has arrived."""
            m16a = small.tile([P, 16], F32, name="m16a", tag="m16a", bufs=2)
            nc.vector.max(out=m16a[:, 0:8], in_=S[:, 0:RA2])
            nc.vector.max(out=m16a[:, 8:16], in_=S[:, RA2:RA])
            m1 = small.tile([P, 8], F32, name="m1", tag="m1", bufs=2)
            nc.vector.max(out=m1[:], in_=m16a[:])
            nc.vector.match_replace(
                out=S[:, RA:2 * RA], in_to_replace=m1[:], in_values=S[:, 0:RA],
                imm_value=-1e30,
            )
            m2 = small.tile([P, 8], F32, name="m2", tag="m2", bufs=2)
            nc.vector.max(out=m2[:], in_=S[:, RA:2 * RA])
            # one mask half on the idle Pool engine, one on the DVE: full-Pool
            # masks would extend S's lifetime past the DVE block period and
            # stall the S buffer rotation.
            halves = []
            for h in range(2):
                mh = maskp.tile([P, RA2], BF16, name="mh", tag="mh", bufs=2)
                nc.gpsimd.tensor_scalar(
                    out=mh[:], in0=S[:, h * RA2:(h + 1) * RA2],
                    scalar1=m2[:, kA - 9:kA - 8], scalar2=None, op0=ALU.is_ge,
                )
                halves.append(mh)
            return halves

        def a_counts(b, halves):
            """Counts + mode for A query block b given its mask halves."""
            psc = psC_p.tile([P, C], F32, name="psC", tag="psC")
            GT = 8  # transposes batched per PSUM bank / ACT copy
            for i0 in range(0, RTA, GT):
                pst = psT_p.tile([P, GT * P], BF16, name="psT", tag="psT")
                for u in range(GT):
                    i = i0 + u
                    mh = halves[i // HTA]
                    lo = (i % HTA) * P
                    nc.tensor.transpose(
                        pst[:, u * P:(u + 1) * P], mh[:, lo:lo + P], identb[:]
                    )
                mTg = maskp.tile([P, GT * P], BF16, name="mTg", tag="mTg", bufs=1)
                nc.scalar.copy(mTg[:], pst[:])
                for u in range(GT):
                    i = i0 + u
                    nc.tensor.matmul(
                        psc[:],
                        mTg[:, u * P:(u + 1) * P],
                        yoht[:, i * C:(i + 1) * C],
                        start=(i == 0),
                        stop=(i == RTA - 1),
                    )
            counts = small.tile([P, C], F32, name="counts", tag="counts")
            nc.scalar.copy(counts[:], psc[:])
            # mode = first argmax of counts
            maxc = small.tile([P, 1], F32, name="maxc", tag="maxc")
            nc.vector.reduce_max(maxc[:], counts[:], axis=AX.X)
            lt01 = small.tile([P, C], F32, name="lt01", tag="lt01")
            nc.vector.tensor_scalar(
                out=lt01[:], in0=counts[:], scalar1=maxc[:], scalar2=None,
                op0=ALU.is_lt,
            )
            cand = small.tile([P, C], F32, name="cand", tag="cand")
            nc.vector.scalar_tensor_tensor(
                out=cand[:], in0=lt01[:], scalar=1e9, in1=iot[:],
                op0=ALU.mult, op1=ALU.add,
            )
            ym = small.tile([P, 1], F32, name="ym", tag="ym")
            nc.vector.tensor_reduce(ym[:], cand[:], axis=AX.X, op=ALU.min)
            nc.sync.dma_start(ym_ap[b], ym[:])

        R2 = R // 2

        def b_max(b, S):
            """Top-8 values (two half scans + merge) and their indices."""
            m16 = small.tile([P, 16], F32, name="m16", tag="m16", bufs=2)
            nc.vector.max(out=m16[:, 0:8], in_=S[:, 0:R2])
            nc.vector.max(out=m16[:, 8:16], in_=S[:, R2:R])
            m8 = small.tile([P, 8], F32, name="m8", tag="m8", bufs=2)
            nc.vector.max(out=m8[:], in_=m16[:])
            idx16 = small.tile([P, 8], U16, name="idx16", tag="idx16", bufs=2)
            nc.vector.max_index(idx16[:], m8[:], S[:])
            nc.sync.dma_start(idx_ap[b], idx16[:])

        # A blocks first (their DVE work covers the xcT DMA tail), then B
        # blocks; counts for A block b are emitted after block b+1's
        # threshold so PE counts work overlaps the DVE threshold tail.
        pending = None
        for b in range(QAB):
            S = scores(b, qTa, QA, RCHA)
            halves = a_thresh(b, S)
            if pending is not None:
                a_counts(*pending)
            pending = (b, halves)
        Sb0 = scores(0, qTb, QB, RCHB)
        a_counts(*pending)
        for b in range(QBB):
            Sn = scores(b + 1, qTb, QB, RCHB) if b + 1 < QBB else None
            b_max(b, Sb0)
            Sb0 = Sn
    nc.compile()
    return nc


def build_gm(Q, D, C, n_cores=NCORES):
    """GM launch: per-row gaussian-mixture loss against per-class means.

    Small enough to be latency-bound, so the post-exp arithmetic is batched
    across all QBB query blocks as wide [P, QBB*C] DVE ops; only the ops
    that need a per-(block, partition) scalar (exp bias, onehot, normalize,
    square+reduce) stay per-block.
    """
    DCH, QBB = D // P, Q // P
    nc = bacc.Bacc(
        "TRN2", target_bir_lowering=False, debug=False, num_devices=n_cores
    )
    qT_ap = nc.dram_tensor("qT", [P, DCH * Q], BF16, kind="ExternalInput").ap()
    muT_ap = nc.dram_tensor("muT", [P, DCH * C], BF16, kind="ExternalInput").ap()
    # emu replicated across blocks: [P, QBB*C]
    emu_ap = nc.dram_tensor("emu", [P, QBB * C], F32, kind="ExternalInput").ap()
    # qaux col b = own labels of block b; col QBB+b = -aa/2 (exp bias)
    qaux_ap = nc.dram_tensor("qaux", [P, 2 * QBB], F32, kind="ExternalInput").ap()
    io_ap = nc.dram_tensor("iotaf", [P, C], F32, kind="ExternalInput").ap()
    lg_ap = nc.dram_tensor("lgm", [P, QBB], F32, kind="ExternalOutput").ap()

    with tile.TileContext(nc) as tc, ExitStack() as ctx:
        consts = ctx.enter_context(tc.tile_pool(name="consts", bufs=1))
        small = ctx.enter_context(tc.tile_pool(name="small", bufs=1))
        psG_p = ctx.enter_context(tc.tile_pool(name="psG", bufs=2, space="PSUM"))

        tchV = consts.tile([1, 1], F32, name="tchV", tag="tchV")
        tchA = consts.tile([1, 1], F32, name="tchA", tag="tchA")
        muTt = consts.tile([P, DCH * C], BF16, name="muTt", tag="muTt")
        nc.sync.dma_start(muTt[:], muT_ap[:])
        qTt = consts.tile([P, DCH * Q], BF16, name="qTt", tag="qTt")
        nc.sync.dma_start(qTt[:], qT_ap[:])
        emut = consts.tile([P, QBB, C], F32, name="emut", tag="emut")
        nc.sync.dma_start(emut[:], emu_ap[:])
        qauxt = consts.tile([P, 2 * QBB], F32, name="qauxt", tag="qauxt")
        nc.sync.dma_start(qauxt[:], qaux_ap[:])
        iot = consts.tile([P, C], F32, name="iot", tag="iot")
        nc.sync.dma_start(iot[:], io_ap[:])
        nc.vector.tensor_copy(tchV[:], emut[0:1, 0:1, 0:1])
        nc.vector.tensor_copy(tchV[:], qauxt[0:1, 0:1])
        nc.vector.tensor_copy(tchV[:], iot[0:1, 0:1])
        nc.scalar.copy(tchA[:], qauxt[0:1, 0:1])

        eg_all = small.tile([P, QBB, C], F32, name="eg_all", tag="eg_all")
        yh_all = small.tile([P, QBB, C], F32, name="yh_all", tag="yh_all")
        for b in range(QBB):
            psg = psG_p.tile([P, C], F32, name="psG", tag="psG")
            for d in range(DCH):
                nc.tensor.matmul(
                    psg[:],
                    qTt[:, d * Q + b * P: d * Q + (b + 1) * P],
                    muTt[:, d * C:(d + 1) * C],
                    start=(d == 0),
                    stop=(d == DCH - 1),
                )
            nc.scalar.activation(
                eg_all[:, b, :], psg[:], mybir.ActivationFunctionType.Exp,
                bias=qauxt[:, QBB + b:QBB + b + 1], scale=1.0,
            )
            nc.vector.tensor_scalar(
                out=yh_all[:, b, :], in0=iot[:], scalar1=qauxt[:, b:b + 1],
                scalar2=None, op0=ALU.is_equal,
            )
        piu_all = small.tile([P, QBB, C], F32, name="piu_all", tag="piu_all")
        nc.vector.tensor_mul(piu_all[:], eg_all[:], emut[:])
        srow8 = small.tile([P, QBB], F32, name="srow8", tag="srow8")
        nc.vector.reduce_sum(srow8[:], piu_all[:], axis=AX.X)
        nc.vector.tensor_scalar_add(srow8[:], srow8[:], 1e-15)
        rec8 = small.tile([P, QBB], F32, name="rec8", tag="rec8")
        nc.vector.reciprocal(rec8[:], srow8[:])
        lg8 = small.tile([P, QBB], F32, name="lg8", tag="lg8")
        for b in range(QBB):
            diff = small.tile([P, C], F32, name="diff", tag="diff", bufs=2)
            nc.vector.scalar_tensor_tensor(
                out=diff[:], in0=piu_all[:, b, :], scalar=rec8[:, b:b + 1],
                in1=yh_all[:, b, :], op0=ALU.mult, op1=ALU.subtract,
            )
            sqj = small.tile([P, C], F32, name="sqj", tag="sqj", bufs=2)
            nc.vector.tensor_mul(sqj[:], diff[:], diff[:])
            nc.vector.reduce_sum(lg8[:, b:b + 1], sqj[:], axis=AX.X)
        nc.sync.dma_start(lg_ap[:], lg8[:])
    nc.compile()
    return nc


# ---------------- host-side packing helpers ----------------

def pack_T(m):
    """[R, D] fp32 -> bf16 [P, (D//P)*R]: column block d holds rows d*P..(d+1)*P
    of m.T (i.e. element (p, d*R + r) = m[r, d*P + p])."""
    R, D = m.shape
    DCH = D // P
    mt = np.ascontiguousarray(m.T.astype(BF16_NP))  # [D, R]
    return np.ascontiguousarray(
        mt.reshape(DCH, P, R).transpose(1, 0, 2).reshape(P, DCH * R)
    )


def pack_bbhl(bb):
    """[R] fp32 -> [2, R] bf16 hi/lo split of -bb/2 (exact to ~2^-17 rel)."""
    t = (-0.5 * bb).astype(np.float32)
    hi = t.astype(BF16_NP)
    lo = (t - hi.astype(np.float32)).astype(BF16_NP)
    return np.ascontiguousarray(np.stack([hi, lo]))


def pack_cols(v):
    """[Q] -> [P, Q//P] fp32: column b = v[b*P:(b+1)*P]."""
    QB = v.shape[0] // P
    return np.ascontiguousarray(v.reshape(QB, P).T.astype(np.float32))


def mode_rows_host(vals):
    """[M, K] labels -> [M] torch.mode semantics (most frequent, smallest on
    ties)."""
    eq = vals[:, :, None] == vals[:, None, :]
    counts = eq.sum(axis=2)
    maxc = counts.max(axis=1, keepdims=True)
    masked = np.where(counts == maxc, vals, np.inf)
    return masked.min(axis=1)


_PROGRAMS = {}
LAST_EXEC_NS = None
_EXEC_NS = {}


def _get_program(key, builder):
    if key not in _PROGRAMS:
        _PROGRAMS[key] = builder()
    return _PROGRAMS[key]


def _run(nc, in_maps, phase):
    import os

    kwargs = {}
    if os.environ.get("KERNEL_TRACE"):
        kwargs = dict(trace=True, trace_cores=[0])
    t0 = _time.time()
    res = run_bass_kernel_spmd(
        nc, in_maps, core_ids=list(range(NCORES)), **kwargs
    )
    if os.environ.get("KERNEL_TIME"):
        print(f"phase {phase} dispatch+exec: {_time.time() - t0:.3f}s")
    if res.exec_time_ns:
        _EXEC_NS[phase] = res.exec_time_ns
        if res.instructions_and_trace:
            print(f"phase {phase}: {res.exec_time_ns} ns, "
                  f"trace: {res.instructions_and_trace[1]}")
    global LAST_EXEC_NS
    if len(_EXEC_NS) == 2:
        LAST_EXEC_NS = sum(_EXEC_NS.values())
    return res


def kernel(x, y, lam, perm):
    x = np.asarray(x, dtype=np.float32)
    y = np.asarray(y, dtype=np.float32)
    lam = np.float32(np.asarray(lam))
    perm = np.asarray(perm, dtype=np.int32)
    N, D = x.shape
    C = CLASSES
    x_ul = (x * lam + x[perm] * (np.float32(1.0) - lam)).astype(np.float32)
    xc = np.concatenate([x, x_ul], axis=0)
    num = xc.shape[0]

    iota_in = np.ascontiguousarray(
        np.broadcast_to(np.arange(C, dtype=np.float32), (P, C))
    )

    # ---------------- launch K: both kNN problems ----------------
    QA = N // NCORES
    QB_ = num // NCORES
    ncK = _get_program(
        ("K", num, N, QA, QB_, D),
        lambda: build_knn(num, N, QA, QB_, D, C, 11),
    )
    aa = (xc.astype(np.float64) ** 2).sum(1).astype(np.float32)
    xcT_in = pack_T(xc).reshape(P, D // P, num)
    bb_in = pack_bbhl(aa)
    ylab_in = pack_cols(y)
    in_maps = []
    for c in range(NCORES):
        in_maps.append(
            {
                "xcT": xcT_in,
                "qTa": pack_T(x_ul[c * QA:(c + 1) * QA]),
                "qTb": pack_T(xc[c * QB_:(c + 1) * QB_]),
                "bbhl": bb_in,
                "ylab": ylab_in,
                "iotaf": iota_in,
            }
        )
    resK = _run(ncK, in_maps, "K")
    y_ul = np.concatenate(
        [r["ymode"].reshape(QA) for r in resK.results]
    ).astype(np.float32)
    # idxo[b, p, j] = j-th nearest ref of query (b*128 + p); rank 0 is self.
    idx_all = np.concatenate(
        [r["idxo"].reshape(QB_, 8) for r in resK.results]
    ).astype(np.int64)

    # ---------------- host glue: per-class means, 3-NN mode ----------------
    yc = np.concatenate([y, y_ul], axis=0)
    yi = yc.astype(np.int32)
    counts = np.bincount(yi, minlength=C).astype(np.float32)
    mu = np.zeros((C, D), dtype=np.float32)
    np.add.at(mu, yi, xc)
    mu = mu / np.maximum(counts, 1.0)[:, None]
    bbm = (mu.astype(np.float64) ** 2).sum(1)
    emu = (np.exp(-bbm / 2.0) * (counts > 0)).astype(np.float32)
    QBB = (num // NCORES) // P
    emu_in = np.ascontiguousarray(
        np.broadcast_to(np.tile(emu, QBB), (P, QBB * C))
    )
    muT_in = pack_T(mu)
    y_ng = mode_rows_host(yc[idx_all[:, 1:4]]).astype(np.float32)

    # ---------------- launch G: gm loss rows ----------------
    ncG = _get_program(("G", QB_, D), lambda: build_gm(QB_, D, C))
    in_maps = []
    for c in range(NCORES):
        sl = slice(c * QB_, (c + 1) * QB_)
        qaux = np.concatenate(
            [pack_cols(yc[sl]), pack_cols(-0.5 * aa[sl])], axis=1
        ).astype(np.float32)
        in_maps.append(
            {
                "qT": pack_T(xc[c * QB_:(c + 1) * QB_]),
                "muT": muT_in,
                "emu": emu_in,
                "qaux": np.ascontiguousarray(qaux),
                "iotaf": iota_in,
            }
        )
    resG = _run(ncG, in_maps, "G")
    # lgm[p, b] = per-row loss of query (b*128 + p) on that core
    lgm_rows = np.concatenate(
        [r["lgm"].reshape(P, QB_ // P).T.reshape(QB_) for r in resG.results]
    )

    loss_gm = np.float32(lgm_rows.mean(dtype=np.float64))
    loss_knn = np.float32(((y_ng - yc) ** 2).mean(dtype=np.float64))
    return np.float32(loss_gm + np.float32(0.01) * loss_knn)


# revision 26
# speedup vs baseline: 1.4579x; 1.0048x over previous
"""Trainium2 Bass kernel for nn_DGMMLoss (retrieval_knn).

Reference computation (see problem statement):
  1. x_ul = lam*x + (1-lam)*x[perm]; pseudo-label via mode of 11-NN labels
  2. concat; per-class means; gaussian-mixture loss term
  3. kNN regularizer: mode of 3-NN (self-excluded) labels, MSE
  loss = loss_gm + 0.01 * loss_knn

Device strategy (8 NeuronCores, data-parallel over query rows; two SPMD
launches):

Launch K (one program, ~all the FLOPs): both kNN problems share the ref set
  xc = [x; x_ul] (phase A only scans the x half), so one 8MB bf16 xcT load
  feeds both. Scores s[q,r] = q.r - ||r||^2/2 via bf16 matmuls (fp32 psum);
  the -bb/2 term rides in the same accumulation as an augmented K=2
  contraction of a ones column against a bf16 hi/lo split (exact to ~2^-17
  rel), so psum evacuation is a pure ACT copy.
  - A-part (11-NN pseudo-labels, 4 query blocks/core): per-row 11th-largest
    via DVE max8+match_replace+max8; tensor_scalar is_ge gives a bf16
    mask[q,r]; per-class counts = maskT.T @ onehot(y) on the PE (mask tiles
    transposed on the PE, batched per PSUM bank, evacuated by ACT); mode =
    first argmax (smallest class on ties, = torch.mode) on DVE.
  - B-part (3-NN indices, 8 query blocks/core): top-8 values from two
    half-row max8s merged by a 16-wide max8; one DVE max_index scan yields
    uint16 column indices, DMA'd to the host. Self is always rank 0 (score
    gap орders above bf16 noise), so the host mode over label ranks 1..3
    reproduces the self-excluded 3-NN mode. No transposes/counts matmuls.
  A blocks are emitted first (their DVE work covers the xcT DMA tail), then
  B blocks, software-pipelined so the DVE never head-blocks.

Launch G (tiny): gaussian-mixture rows. Needs per-class means, which the
  host computes from phase A's pseudo-labels. Per 128-query block: 4 PE
  matmuls q.muT (100 cols), ACT exp(. - aa/2), and a short DVE chain
  (normalize, subtract onehot, fused square+reduce) -> per-row loss.

Host does only O(N*D) glue: x_ul, norms, packing, per-class means,
label-gather + mode-of-3 from device indices, final scalar assembly. bf16
scoring shifts the loss by ~1e-3 relative (verified against an fp64 model;
fp32 matmul on TRN2 is 4x slower than bf16).
"""

from contextlib import ExitStack

import numpy as np
import ml_dtypes

import time as _time

import concourse.bacc as bacc
import concourse.tile as tile
import concourse.mybir as mybir
from concourse.bass_utils import run_bass_kernel_spmd
from concourse.masks import make_identity

P = 128
NCORES = 8
CLASSES = 100
F32 = mybir.dt.float32
BF16 = mybir.dt.bfloat16
U16 = mybir.dt.uint16
BF16_NP = ml_dtypes.bfloat16
ALU = mybir.AluOpType
AX = mybir.AxisListType


def build_knn(R, RA, QA, QB, D, C, kA, n_cores=NCORES):
    """Merged kNN launch: A-part = 11-NN mode over the first RA refs for QA
    queries; B-part = top-8 neighbor indices over all R refs for QB queries.
    """
    DCH = D // P
    RTA, RCHA, QAB = RA // P, RA // 512, QA // P
    RCHB, QBB = R // 512, QB // P
    assert D % P == 0 and R % 1024 == 0 and RA % 1024 == 0 and 8 < kA <= 16

    nc = bacc.Bacc(
        "TRN2", target_bir_lowering=False, debug=False, num_devices=n_cores
    )
    xT_ap = nc.dram_tensor("xcT", [P, DCH, R], BF16, kind="ExternalInput").ap()
    qa_ap = nc.dram_tensor("qTa", [P, DCH * QA], BF16, kind="ExternalInput").ap()
    qb_ap = nc.dram_tensor("qTb", [P, DCH * QB], BF16, kind="ExternalInput").ap()
    bb_ap = nc.dram_tensor("bbhl", [2, R], BF16, kind="ExternalInput").ap()
    yl_ap = nc.dram_tensor("ylab", [P, RTA], F32, kind="ExternalInput").ap()
    io_ap = nc.dram_tensor("iotaf", [P, C], F32, kind="ExternalInput").ap()
    ym_ap = nc.dram_tensor("ymode", [QAB, P, 1], F32, kind="ExternalOutput").ap()
    idx_ap = nc.dram_tensor("idxo", [QBB, P, 8], U16, kind="ExternalOutput").ap()

    with tile.TileContext(nc) as tc, ExitStack() as ctx:
        consts = ctx.enter_context(tc.tile_pool(name="consts", bufs=1))
        sbig = ctx.enter_context(tc.tile_pool(name="sbig", bufs=3))
        maskp = ctx.enter_context(tc.tile_pool(name="maskp", bufs=1))
        small = ctx.enter_context(tc.tile_pool(name="small", bufs=1))
        psS_p = ctx.enter_context(tc.tile_pool(name="psS", bufs=3, space="PSUM"))
        psT_p = ctx.enter_context(tc.tile_pool(name="psT", bufs=2, space="PSUM"))
        psC_p = ctx.enter_context(tc.tile_pool(name="psC", bufs=1, space="PSUM"))

        identb = consts.tile([P, P], BF16, name="identb", tag="identb")
        make_identity(nc, identb)

        # Tiny "touch" ops absorb DMA-queue waits into dedicated copies so the
        # wide compute instructions (1-2 HW wait slots) only wait on engine
        # semaphores.
        tchV = consts.tile([1, 1], F32, name="tchV", tag="tchV")

        def dve_touch(ap):
            nc.vector.tensor_copy(tchV[:], ap[0:1, 0:1])

        # PE touch of the identity so later transposes don't carry its wait.
        psI = psT_p.tile([1, P], BF16, name="psI", tag="psMI", bufs=1)
        nc.tensor.transpose(psI[:], identb[:, 0:1], identb[:])

        # DMA constants in; small/label-side tiles first so the yoht build and
        # the A-part aren't gated on the full xcT load; qTb (B-part only)
        # after the A-part ref groups. Each ref group is ONE strided DMA of
        # all DCH d-slices (DMA issue costs ~650ns each; fewer is faster).
        GROUP = 1024
        NG = R // GROUP
        NGA = RA // GROUP
        ylabt = consts.tile([P, RTA], F32, name="ylabt", tag="ylabt")
        nc.sync.dma_start(ylabt[:], yl_ap[:])
        iot = consts.tile([P, C], F32, name="iot", tag="iot")
        nc.sync.dma_start(iot[:], io_ap[:])
        qTa = consts.tile([P, DCH * QA], BF16, name="qTa", tag="qTa")
        nc.sync.dma_start(qTa[:], qa_ap[:])
        bbt = consts.tile([2, R], BF16, name="bbt", tag="bbt")
        nc.sync.dma_start(bbt[:], bb_ap[:])
        ones2 = consts.tile([2, P], BF16, name="ones2", tag="ones2")
        nc.vector.memset(ones2[:], 1.0)
        xgs = [None] * NG
        qTb = consts.tile([P, DCH * QB], BF16, name="qTb", tag="qTb")

        def load_group(g):
            t = consts.tile([P, DCH, GROUP], BF16, name=f"xg{g}", tag=f"xg{g}")
            nc.sync.dma_start(t[:], xT_ap[:, :, g * GROUP:(g + 1) * GROUP])
            xgs[g] = t

        for g in range(NGA):
            load_group(g)
        nc.sync.dma_start(qTb[:], qb_ap[:])
        for g in range(NGA, NG):
            load_group(g)
        dve_touch(iot)
        dve_touch(ylabt)
        # one-hot labels built on device: yoht[:, i*C:(i+1)*C] = (iota == y_r).
        # Runs on the otherwise-idle Pool engine to keep the DVE free.
        yoht = consts.tile([P, RTA * C], BF16, name="yoht", tag="yoht")
        for i in range(RTA):
            nc.gpsimd.tensor_scalar(
                out=yoht[:, i * C:(i + 1) * C], in0=iot[:],
                scalar1=ylabt[:, i:i + 1], scalar2=None, op0=ALU.is_equal,
            )

        def scores(b, qt, Qtot, rch):
            """S[q, r] = q.r - bb_r/2 for query block b (queries from qt)."""
            S = sbig.tile([P, R], F32, name="S", tag="S")
            for j in range(rch):
                g, go = (j * 512) // GROUP, (j * 512) % GROUP
                ps = psS_p.tile([P, 512], F32, name="psS", tag="psS")
                for d in range(DCH):
                    nc.tensor.matmul(
                        ps[:],
                        qt[:, d * Qtot + b * P: d * Qtot + (b + 1) * P],
                        xgs[g][:, d, go:go + 512],
                        start=(d == 0),
                        stop=False,
                    )
                nc.tensor.matmul(
                    ps[:],
                    ones2[:],
                    bbt[:, j * 512:(j + 1) * 512],
                    start=False,
                    stop=True,
                )
                nc.scalar.copy(S[:, j * 512:(j + 1) * 512], ps[:])
            return S

        RA2 = RA // 2
        HTA = RTA // 2  # A-part mask tiles per half

        def a_thresh(b, S):
            """11th-largest threshold + bf16 mask halves for A block b.

            The A-part only scans refs [0, RA), so the upper half of its
            [P, R] S tile is free scratch for the match_replace output. The
            first max8 runs as two half scans + merge so block 0's first scan
            starts as soon as the first half of the refs has arrived."""
            m16a = small.tile([P, 16], F32, name="m16a", tag="m16a", bufs=2)
            nc.vector.max(out=m16a[:, 0:8], in_=S[:, 0:RA2])
            nc.vector.max(out=m16a[:, 8:16], in_=S[:, RA2:RA])
            m1 = small.tile([P, 8], F32, name="m1", tag="m1", bufs=2)
            nc.vector.max(out=m1[:], in_=m16a[:])
            nc.vector.match_replace(
                out=S[:, RA:2 * RA], in_to_replace=m1[:], in_values=S[:, 0:RA],
                imm_value=-1e30,
            )
            m2 = small.tile([P, 8], F32, name="m2", tag="m2", bufs=2)
            nc.vector.max(out=m2[:], in_=S[:, RA:2 * RA])
            # one mask half on the idle Pool engine, one on the DVE: full-Pool
            # masks would extend S's lifetime past the DVE block period and
            # stall the S buffer rotation.
            halves = []
            for h in range(2):
                mh = maskp.tile([P, RA2], BF16, name="mh", tag="mh", bufs=2)
                nc.gpsimd.tensor_scalar(
                    out=mh[:], in0=S[:, h * RA2:(h + 1) * RA2],
                    scalar1=m2[:, kA - 9:kA - 8], scalar2=None, op0=ALU.is_ge,
                )
                halves.append(mh)
            return halves

        def a_counts(b, halves):
            """Counts + mode for A query block b given its mask halves."""
            psc = psC_p.tile([P, C], F32, name="psC", tag="psC")
            GT = 8  # transposes batched per PSUM bank / ACT copy
            for i0 in range(0, RTA, GT):
                pst = psT_p.tile([P, GT * P], BF16, name="psT", tag="psT")
                for u in range(GT):
                    i = i0 + u
                    mh = halves[i // HTA]
                    lo = (i % HTA) * P
                    nc.tensor.transpose(
                        pst[:, u * P:(u + 1) * P], mh[:, lo:lo + P], identb[:]
                    )
                mTg = maskp.tile([P, GT * P], BF16, name="mTg", tag="mTg", bufs=1)
                nc.scalar.copy(mTg[:], pst[:])
                for u in range(GT):
                    i = i0 + u
                    nc.tensor.matmul(
                        psc[:],
                        mTg[:, u * P:(u + 1) * P],
                        yoht[:, i * C:(i + 1) * C],
                        start=(i == 0),
                        stop=(i == RTA - 1),
                    )
            counts = small.tile([P, C], F32, name="counts", tag="counts")
            nc.scalar.copy(counts[:], psc[:])
            # mode = first argmax of counts
            maxc = small.tile([P, 1], F32, name="maxc", tag="maxc")
            nc.vector.reduce_max(maxc[:], counts[:], axis=AX.X)
            lt01 = small.tile([P, C], F32, name="lt01", tag="lt01")
            nc.vector.tensor_scalar(
                out=lt01[:], in0=counts[:], scalar1=maxc[:], scalar2=None,
                op0=ALU.is_lt,
            )
            cand = small.tile([P, C], F32, name="cand", tag="cand")
            nc.vector.scalar_tensor_tensor(
                out=cand[:], in0=lt01[:], scalar=1e9, in1=iot[:],
                op0=ALU.mult, op1=ALU.add,
            )
            ym = small.tile([P, 1], F32, name="ym", tag="ym")
            nc.vector.tensor_reduce(ym[:], cand[:], axis=AX.X, op=ALU.min)
            nc.sync.dma_start(ym_ap[b], ym[:])

        R2 = R // 2

        def b_max(b, S):
            """Top-8 values (two half scans + merge) and their indices."""
            m16 = small.tile([P, 16], F32, name="m16", tag="m16", bufs=2)
            nc.vector.max(out=m16[:, 0:8], in_=S[:, 0:R2])
            nc.vector.max(out=m16[:, 8:16], in_=S[:, R2:R])
            m8 = small.tile([P, 8], F32, name="m8", tag="m8", bufs=2)
            nc.vector.max(out=m8[:], in_=m16[:])
            idx16 = small.tile([P, 8], U16, name="idx16", tag="idx16", bufs=2)
            nc.vector.max_index(idx16[:], m8[:], S[:])
            nc.sync.dma_start(idx_ap[b], idx16[:])

        # A blocks first (their DVE work covers the xcT DMA tail), then B
        # blocks; counts for A block b are emitted after block b+1's
        # threshold so PE counts work overlaps the DVE threshold tail.
        pending = None
        for b in range(QAB):
            S = scores(b, qTa, QA, RCHA)
            halves = a_thresh(b, S)
            if pending is not None:
                a_counts(*pending)
            pending = (b, halves)
        Sb0 = scores(0, qTb, QB, RCHB)
        a_counts(*pending)
        for b in range(QBB):
            Sn = scores(b + 1, qTb, QB, RCHB) if b + 1 < QBB else None
            b_max(b, Sb0)
            Sb0 = Sn
    nc.compile()
    return nc


def build_gm(Q, D, C, n_cores=NCORES):
    """GM launch: per-row gaussian-mixture loss against per-class means.

    Small enough to be latency-bound, so the post-exp arithmetic is batched
    across all QBB query blocks as wide [P, QBB*C] DVE ops; only the ops
    that need a per-(block, partition) scalar (exp bias, onehot, normalize,
    square+reduce) stay per-block.
    """
    DCH, QBB = D // P, Q // P
    nc = bacc.Bacc(
        "TRN2", target_bir_lowering=False, debug=False, num_devices=n_cores
    )
    qT_ap = nc.dram_tensor("qT", [P, DCH * Q], BF16, kind="ExternalInput").ap()
    muT_ap = nc.dram_tensor("muT", [P, DCH * C], BF16, kind="ExternalInput").ap()
    # emu replicated across blocks: [P, QBB*C]
    emu_ap = nc.dram_tensor("emu", [P, QBB * C], F32, kind="ExternalInput").ap()
    # qaux col b = own labels of block b; col QBB+b = -aa/2 (exp bias)
    qaux_ap = nc.dram_tensor("qaux", [P, 2 * QBB], F32, kind="ExternalInput").ap()
    io_ap = nc.dram_tensor("iotaf", [P, C], F32, kind="ExternalInput").ap()
    lg_ap = nc.dram_tensor("lgm", [P, QBB], F32, kind="ExternalOutput").ap()

    with tile.TileContext(nc) as tc, ExitStack() as ctx:
        consts = ctx.enter_context(tc.tile_pool(name="consts", bufs=1))
        small = ctx.enter_context(tc.tile_pool(name="small", bufs=1))
        psG_p = ctx.enter_context(tc.tile_pool(name="psG", bufs=2, space="PSUM"))

        tchV = consts.tile([1, 1], F32, name="tchV", tag="tchV")
        tchA = consts.tile([1, 1], F32, name="tchA", tag="tchA")
        qTt = consts.tile([P, DCH * Q], BF16, name="qTt", tag="qTt")
        nc.sync.dma_start(qTt[:], qT_ap[:])
        muTt = consts.tile([P, DCH * C], BF16, name="muTt", tag="muTt")
        nc.sync.dma_start(muTt[:], muT_ap[:])
        qauxt = consts.tile([P, 2 * QBB], F32, name="qauxt", tag="qauxt")
        nc.sync.dma_start(qauxt[:], qaux_ap[:])
        iot = consts.tile([P, C], F32, name="iot", tag="iot")
        nc.sync.dma_start(iot[:], io_ap[:])
        emut = consts.tile([P, QBB, C], F32, name="emut", tag="emut")
        nc.sync.dma_start(emut[:], emu_ap[:])
        nc.vector.tensor_copy(tchV[:], qauxt[0:1, 0:1])
        nc.vector.tensor_copy(tchV[:], iot[0:1, 0:1])
        nc.vector.tensor_copy(tchV[:], emut[0:1, 0:1, 0:1])
        nc.scalar.copy(tchA[:], qauxt[0:1, 0:1])

        eg_all = small.tile([P, QBB, C], F32, name="eg_all", tag="eg_all")
        yh_all = small.tile([P, QBB, C], F32, name="yh_all", tag="yh_all")
        # onehots only need qaux+iota: run during the qT/muT DMA fill
        for b in range(QBB):
            nc.vector.tensor_scalar(
                out=yh_all[:, b, :], in0=iot[:], scalar1=qauxt[:, b:b + 1],
                scalar2=None, op0=ALU.is_equal,
            )
        for b in range(QBB):
            psg = psG_p.tile([P, C], F32, name="psG", tag="psG")
            for d in range(DCH):
                nc.tensor.matmul(
                    psg[:],
                    qTt[:, d * Q + b * P: d * Q + (b + 1) * P],
                    muTt[:, d * C:(d + 1) * C],
                    start=(d == 0),
                    stop=(d == DCH - 1),
                )
            nc.scalar.activation(
                eg_all[:, b, :], psg[:], mybir.ActivationFunctionType.Exp,
                bias=qauxt[:, QBB + b:QBB + b + 1], scale=1.0,
            )
        piu_all = small.tile([P, QBB, C], F32, name="piu_all", tag="piu_all")
        nc.vector.tensor_mul(piu_all[:], eg_all[:], emut[:])
        srow8 = small.tile([P, QBB], F32, name="srow8", tag="srow8")
        nc.vector.reduce_sum(srow8[:], piu_all[:], axis=AX.X)
        nc.vector.tensor_scalar_add(srow8[:], srow8[:], 1e-15)
        rec8 = small.tile([P, QBB], F32, name="rec8", tag="rec8")
        nc.vector.reciprocal(rec8[:], srow8[:])
        lg8 = small.tile([P, QBB], F32, name="lg8", tag="lg8")
        for b in range(QBB):
            diff = small.tile([P, C], F32, name="diff", tag="diff", bufs=2)
            nc.vector.scalar_tensor_tensor(
                out=diff[:], in0=piu_all[:, b, :], scalar=rec8[:, b:b + 1],
                in1=yh_all[:, b, :], op0=ALU.mult, op1=ALU.subtract,
            )
            sqj = small.tile([P, C], F32, name="sqj", tag="sqj", bufs=2)
            nc.vector.tensor_mul(sqj[:], diff[:], diff[:])
            nc.vector.reduce_sum(lg8[:, b:b + 1], sqj[:], axis=AX.X)
        nc.sync.dma_start(lg_ap[:], lg8[:])
    nc.compile()
    return nc


# ---------------- host-side packing helpers ----------------

def pack_T(m):
    """[R, D] fp32 -> bf16 [P, (D//P)*R]: column block d holds rows d*P..(d+1)*P
    of m.T (i.e. element (p, d*R + r) = m[r, d*P + p])."""
    R, D = m.shape
    DCH = D // P
    mt = np.ascontiguousarray(m.T.astype(BF16_NP))  # [D, R]
    return np.ascontiguousarray(
        mt.reshape(DCH, P, R).transpose(1, 0, 2).reshape(P, DCH * R)
    )


def pack_bbhl(bb):
    """[R] fp32 -> [2, R] bf16 hi/lo split of -bb/2 (exact to ~2^-17 rel)."""
    t = (-0.5 * bb).astype(np.float32)
    hi = t.astype(BF16_NP)
    lo = (t - hi.astype(np.float32)).astype(BF16_NP)
    return np.ascontiguousarray(np.stack([hi, lo]))


def pack_cols(v):
    """[Q] -> [P, Q//P] fp32: column b = v[b*P:(b+1)*P]."""
    QB = v.shape[0] // P
    return np.ascontiguousarray(v.reshape(QB, P).T.astype(np.float32))


def mode_rows_host(vals):
    """[M, K] labels -> [M] torch.mode semantics (most frequent, smallest on
    ties)."""
    eq = vals[:, :, None] == vals[:, None, :]
    counts = eq.sum(axis=2)
    maxc = counts.max(axis=1, keepdims=True)
    masked = np.where(counts == maxc, vals, np.inf)
    return masked.min(axis=1)


_PROGRAMS = {}
LAST_EXEC_NS = None
_EXEC_NS = {}


def _get_program(key, builder):
    if key not in _PROGRAMS:
        _PROGRAMS[key] = builder()
    return _PROGRAMS[key]


def _run(nc, in_maps, phase):
    import os

    kwargs = {}
    if os.environ.get("KERNEL_TRACE"):
        kwargs = dict(trace=True, trace_cores=[0])
    t0 = _time.time()
    res = run_bass_kernel_spmd(
        nc, in_maps, core_ids=list(range(NCORES)), **kwargs
    )
    if os.environ.get("KERNEL_TIME"):
        print(f"phase {phase} dispatch+exec: {_time.time() - t0:.3f}s")
    if res.exec_time_ns:
        _EXEC_NS[phase] = res.exec_time_ns
        if res.instructions_and_trace:
            print(f"phase {phase}: {res.exec_time_ns} ns, "
                  f"trace: {res.instructions_and_trace[1]}")
    global LAST_EXEC_NS
    if len(_EXEC_NS) == 2:
        LAST_EXEC_NS = sum(_EXEC_NS.values())
    return res


def kernel(x, y, lam, perm):
    x = np.asarray(x, dtype=np.float32)
    y = np.asarray(y, dtype=np.float32)
    lam = np.float32(np.asarray(lam))
    perm = np.asarray(perm, dtype=np.int32)
    N, D = x.shape
    C = CLASSES
    x_ul = (x * lam + x[perm] * (np.float32(1.0) - lam)).astype(np.float32)
    xc = np.concatenate([x, x_ul], axis=0)
    num = xc.shape[0]

    iota_in = np.ascontiguousarray(
        np.broadcast_to(np.arange(C, dtype=np.float32), (P, C))
    )

    # ---------------- launch K: both kNN problems ----------------
    QA = N // NCORES
    QB_ = num // NCORES
    ncK = _get_program(
        ("K", num, N, QA, QB_, D),
        lambda: build_knn(num, N, QA, QB_, D, C, 11),
    )
    aa = (xc.astype(np.float64) ** 2).sum(1).astype(np.float32)
    xcT_in = pack_T(xc).reshape(P, D // P, num)
    bb_in = pack_bbhl(aa)
    ylab_in = pack_cols(y)
    in_maps = []
    for c in range(NCORES):
        in_maps.append(
            {
                "xcT": xcT_in,
                "qTa": pack_T(x_ul[c * QA:(c + 1) * QA]),
                "qTb": pack_T(xc[c * QB_:(c + 1) * QB_]),
                "bbhl": bb_in,
                "ylab": ylab_in,
                "iotaf": iota_in,
            }
        )
    resK = _run(ncK, in_maps, "K")
    y_ul = np.concatenate(
        [r["ymode"].reshape(QA) for r in resK.results]
    ).astype(np.float32)
    # idxo[b, p, j] = j-th nearest ref of query (b*128 + p); rank 0 is self.
    idx_all = np.concatenate(
        [r["idxo"].reshape(QB_, 8) for r in resK.results]
    ).astype(np.int64)

    # ---------------- host glue: per-class means, 3-NN mode ----------------
    yc = np.concatenate([y, y_ul], axis=0)
    yi = yc.astype(np.int32)
    counts = np.bincount(yi, minlength=C).astype(np.float32)
    mu = np.zeros((C, D), dtype=np.float32)
    np.add.at(mu, yi, xc)
    mu = mu / np.maximum(counts, 1.0)[:, None]
    bbm = (mu.astype(np.float64) ** 2).sum(1)
    emu = (np.exp(-bbm / 2.0) * (counts > 0)).astype(np.float32)
    QBB = (num // NCORES) // P
    emu_in = np.ascontiguousarray(
        np.broadcast_to(np.tile(emu, QBB), (P, QBB * C))
    )
    muT_in = pack_T(mu)
    y_ng = mode_rows_host(yc[idx_all[:, 1:4]]).astype(np.float32)

    # ---------------- launch G: gm loss rows ----------------
    ncG = _get_program(("G", QB_, D), lambda: build_gm(QB_, D, C))
    in_maps = []
    for c in range(NCORES):
        sl = slice(c * QB_, (c + 1) * QB_)
        qaux = np.concatenate(
            [pack_cols(yc[sl]), pack_cols(-0.5 * aa[sl])], axis=1
        ).astype(np.float32)
        in_maps.append(
            {
                "qT": pack_T(xc[c * QB_:(c + 1) * QB_]),
                "muT": muT_in,
                "emu": emu_in,
                "qaux": np.ascontiguousarray(qaux),
                "iotaf": iota_in,
            }
        )
    resG = _run(ncG, in_maps, "G")
    # lgm[p, b] = per-row loss of query (b*128 + p) on that core
    lgm_rows = np.concatenate(
        [r["lgm"].reshape(P, QB_ // P).T.reshape(QB_) for r in resG.results]
    )

    loss_gm = np.float32(lgm_rows.mean(dtype=np.float64))
    loss_knn = np.float32(((y_ng - yc) ** 2).mean(dtype=np.float64))
    return np.float32(loss_gm + np.float32(0.01) * loss_knn)


# revision 31
# speedup vs baseline: 1.4758x; 1.0123x over previous
"""Trainium2 Bass kernel for nn_DGMMLoss (retrieval_knn).

Reference computation (see problem statement):
  1. x_ul = lam*x + (1-lam)*x[perm]; pseudo-label via mode of 11-NN labels
  2. concat; per-class means; gaussian-mixture loss term
  3. kNN regularizer: mode of 3-NN (self-excluded) labels, MSE
  loss = loss_gm + 0.01 * loss_knn

Device strategy (8 NeuronCores, data-parallel over query rows; two SPMD
launches):

Launch K (one program, ~all the FLOPs): both kNN problems share the ref set
  xc = [x; x_ul] (phase A only scans the x half), so one 8MB bf16 xcT load
  feeds both. Scores s[q,r] = q.r - ||r||^2/2 via bf16 matmuls (fp32 psum);
  the -bb/2 term rides in the same accumulation as an augmented K=2
  contraction of a ones column against a bf16 hi/lo split (exact to ~2^-17
  rel), so psum evacuation is a pure ACT copy.
  - A-part (11-NN pseudo-labels, 4 query blocks/core): per-row 11th-largest
    via DVE max8+match_replace+max8; tensor_scalar is_ge gives a bf16
    mask[q,r]; per-class counts = maskT.T @ onehot(y) on the PE (mask tiles
    transposed on the PE, batched per PSUM bank, evacuated by ACT); mode =
    first argmax (smallest class on ties, = torch.mode) on DVE.
  - B-part (3-NN indices, 8 query blocks/core): top-8 values from two
    half-row max8s merged by a 16-wide max8; one DVE max_index scan yields
    uint16 column indices, DMA'd to the host. Self is always rank 0 (score
    gap орders above bf16 noise), so the host mode over label ranks 1..3
    reproduces the self-excluded 3-NN mode. No transposes/counts matmuls.
  A blocks are emitted first (their DVE work covers the xcT DMA tail), then
  B blocks, software-pipelined so the DVE never head-blocks.

Launch G (tiny): gaussian-mixture rows. Needs per-class means, which the
  host computes from phase A's pseudo-labels. Per 128-query block: 4 PE
  matmuls q.muT (100 cols), ACT exp(. - aa/2), and a short DVE chain
  (normalize, subtract onehot, fused square+reduce) -> per-row loss.

Host does only O(N*D) glue: x_ul, norms, packing, per-class means,
label-gather + mode-of-3 from device indices, final scalar assembly. bf16
scoring shifts the loss by ~1e-3 relative (verified against an fp64 model;
fp32 matmul on TRN2 is 4x slower than bf16).
"""

from contextlib import ExitStack

import numpy as np
import ml_dtypes

import time as _time

import concourse.bacc as bacc
import concourse.tile as tile
import concourse.mybir as mybir
from concourse.bass_utils import run_bass_kernel_spmd
from concourse.masks import make_identity

P = 128
NCORES = 8
CLASSES = 100
F32 = mybir.dt.float32
BF16 = mybir.dt.bfloat16
U16 = mybir.dt.uint16
BF16_NP = ml_dtypes.bfloat16
ALU = mybir.AluOpType
AX = mybir.AxisListType


def build_knn(R, RA, QA, QB, D, C, kA, n_cores=NCORES):
    """Merged kNN launch: A-part = 11-NN mode over the first RA refs for QA
    queries; B-part = top-8 neighbor indices over all R refs for QB queries.
    """
    DCH = D // P
    RTA, RCHA, QAB = RA // P, RA // 512, QA // P
    RCHB, QBB = R // 512, QB // P
    assert D % P == 0 and R % 1024 == 0 and RA % 1024 == 0 and 8 < kA <= 16

    nc = bacc.Bacc(
        "TRN2", target_bir_lowering=False, debug=False, num_devices=n_cores
    )
    xT_ap = nc.dram_tensor("xcT", [P, DCH, R], BF16, kind="ExternalInput").ap()
    qa_ap = nc.dram_tensor("qTa", [P, DCH * QA], BF16, kind="ExternalInput").ap()
    qb_ap = nc.dram_tensor("qTb", [P, DCH * QB], BF16, kind="ExternalInput").ap()
    bb_ap = nc.dram_tensor("bbhl", [2, R], BF16, kind="ExternalInput").ap()
    yl_ap = nc.dram_tensor("ylab", [P, RTA], F32, kind="ExternalInput").ap()
    io_ap = nc.dram_tensor("iotaf", [P, C], F32, kind="ExternalInput").ap()
    ym_ap = nc.dram_tensor("ymode", [QAB, P, 1], F32, kind="ExternalOutput").ap()
    idx_ap = nc.dram_tensor("idxo", [QBB, P, 8], U16, kind="ExternalOutput").ap()

    with tile.TileContext(nc) as tc, ExitStack() as ctx:
        consts = ctx.enter_context(tc.tile_pool(name="consts", bufs=1))
        sbig = ctx.enter_context(tc.tile_pool(name="sbig", bufs=3))
        maskp = ctx.enter_context(tc.tile_pool(name="maskp", bufs=1))
        small = ctx.enter_context(tc.tile_pool(name="small", bufs=1))
        psS_p = ctx.enter_context(tc.tile_pool(name="psS", bufs=3, space="PSUM"))
        psT_p = ctx.enter_context(tc.tile_pool(name="psT", bufs=2, space="PSUM"))
        psC_p = ctx.enter_context(tc.tile_pool(name="psC", bufs=1, space="PSUM"))

        identb = consts.tile([P, P], BF16, name="identb", tag="identb")
        make_identity(nc, identb)

        # Tiny "touch" ops absorb DMA-queue waits into dedicated copies so the
        # wide compute instructions (1-2 HW wait slots) only wait on engine
        # semaphores.
        tchV = consts.tile([1, 1], F32, name="tchV", tag="tchV")

        def dve_touch(ap):
            nc.vector.tensor_copy(tchV[:], ap[0:1, 0:1])

        # PE touch of the identity so later transposes don't carry its wait.
        psI = psT_p.tile([1, P], BF16, name="psI", tag="psMI", bufs=1)
        nc.tensor.transpose(psI[:], identb[:, 0:1], identb[:])

        # DMA constants in; small/label-side tiles first so the yoht build and
        # the A-part aren't gated on the full xcT load; qTb (B-part only)
        # after the A-part ref groups. Each ref group is ONE strided DMA of
        # all DCH d-slices (DMA issue costs ~650ns each; fewer is faster).
        GROUP = 1024
        NG = R // GROUP
        NGA = RA // GROUP
        qTa = consts.tile([P, DCH * QA], BF16, name="qTa", tag="qTa")
        nc.sync.dma_start(qTa[:], qa_ap[:])
        bbt = consts.tile([2, R], BF16, name="bbt", tag="bbt")
        nc.sync.dma_start(bbt[:], bb_ap[:])
        ones2 = consts.tile([2, P], BF16, name="ones2", tag="ones2")
        nc.vector.memset(ones2[:], 1.0)
        xgs = [None] * NG
        qTb = consts.tile([P, DCH * QB], BF16, name="qTb", tag="qTb")
        ylabt = consts.tile([P, RTA], F32, name="ylabt", tag="ylabt")
        iot = consts.tile([P, C], F32, name="iot", tag="iot")

        def load_group(g):
            t = consts.tile([P, DCH, GROUP], BF16, name=f"xg{g}", tag=f"xg{g}")
            nc.sync.dma_start(t[:], xT_ap[:, :, g * GROUP:(g + 1) * GROUP])
            xgs[g] = t

        load_group(0)
        # labels/iota only feed the Pool yoht build (first consumed by the
        # counts of block 0, ~40us in) — after the first ref group is enough.
        nc.sync.dma_start(ylabt[:], yl_ap[:])
        nc.sync.dma_start(iot[:], io_ap[:])
        for g in range(1, NGA):
            load_group(g)
        nc.sync.dma_start(qTb[:], qb_ap[:])
        for g in range(NGA, NG):
            load_group(g)
        dve_touch(iot)
        dve_touch(ylabt)
        # one-hot labels built on device: yoht[:, i*C:(i+1)*C] = (iota == y_r).
        # Runs on the otherwise-idle Pool engine to keep the DVE free.
        yoht = consts.tile([P, RTA * C], BF16, name="yoht", tag="yoht")
        for i in range(RTA):
            nc.gpsimd.tensor_scalar(
                out=yoht[:, i * C:(i + 1) * C], in0=iot[:],
                scalar1=ylabt[:, i:i + 1], scalar2=None, op0=ALU.is_equal,
            )

        def scores(b, qt, Qtot, rch):
            """S[q, r] = q.r - bb_r/2 for query block b (queries from qt)."""
            S = sbig.tile([P, R], F32, name="S", tag="S")
            for j in range(rch):
                g, go = (j * 512) // GROUP, (j * 512) % GROUP
                ps = psS_p.tile([P, 512], F32, name="psS", tag="psS")
                for d in range(DCH):
                    nc.tensor.matmul(
                        ps[:],
                        qt[:, d * Qtot + b * P: d * Qtot + (b + 1) * P],
                        xgs[g][:, d, go:go + 512],
                        start=(d == 0),
                        stop=False,
                    )
                nc.tensor.matmul(
                    ps[:],
                    ones2[:],
                    bbt[:, j * 512:(j + 1) * 512],
                    start=False,
                    stop=True,
                )
                nc.scalar.copy(S[:, j * 512:(j + 1) * 512], ps[:])
            return S

        RA2 = RA // 2
        HTA = RTA // 2  # A-part mask tiles per half

        def a_thresh(b, S):
            """11th-largest threshold + bf16 mask halves for A block b.

            The A-part only scans refs [0, RA), so the upper half of its
            [P, R] S tile is free scratch for the match_replace output. The
            first max8 runs as two half scans + merge so block 0's first scan
            starts as soon as the first half of the refs has arrived."""
            RA4 = RA // 4
            m32a = small.tile([P, 32], F32, name="m32a", tag="m32a", bufs=2)
            for qtr in range(4):
                nc.vector.max(
                    out=m32a[:, qtr * 8:(qtr + 1) * 8],
                    in_=S[:, qtr * RA4:(qtr + 1) * RA4],
                )
            m1 = small.tile([P, 8], F32, name="m1", tag="m1", bufs=2)
            nc.vector.max(out=m1[:], in_=m32a[:])
            nc.vector.match_replace(
                out=S[:, RA:2 * RA], in_to_replace=m1[:], in_values=S[:, 0:RA],
                imm_value=-1e30,
            )
            m2 = small.tile([P, 8], F32, name="m2", tag="m2", bufs=2)
            nc.vector.max(out=m2[:], in_=S[:, RA:2 * RA])
            # one mask half on the idle Pool engine, one on the DVE: full-Pool
            # masks would extend S's lifetime past the DVE block period and
            # stall the S buffer rotation.
            halves = []
            for h in range(2):
                mh = maskp.tile([P, RA2], BF16, name="mh", tag="mh", bufs=2)
                nc.gpsimd.tensor_scalar(
                    out=mh[:], in0=S[:, h * RA2:(h + 1) * RA2],
                    scalar1=m2[:, kA - 9:kA - 8], scalar2=None, op0=ALU.is_ge,
                )
                halves.append(mh)
            return halves

        def a_counts(b, halves):
            """Counts + mode for A query block b given its mask halves."""
            psc = psC_p.tile([P, C], F32, name="psC", tag="psC")
            GT = 8  # transposes batched per PSUM bank / ACT copy
            for i0 in range(0, RTA, GT):
                pst = psT_p.tile([P, GT * P], BF16, name="psT", tag="psT")
                for u in range(GT):
                    i = i0 + u
                    mh = halves[i // HTA]
                    lo = (i % HTA) * P
                    nc.tensor.transpose(
                        pst[:, u * P:(u + 1) * P], mh[:, lo:lo + P], identb[:]
                    )
                mTg = maskp.tile([P, GT * P], BF16, name="mTg", tag="mTg", bufs=1)
                nc.scalar.copy(mTg[:], pst[:])
                for u in range(GT):
                    i = i0 + u
                    nc.tensor.matmul(
                        psc[:],
                        mTg[:, u * P:(u + 1) * P],
                        yoht[:, i * C:(i + 1) * C],
                        start=(i == 0),
                        stop=(i == RTA - 1),
                    )
            counts = small.tile([P, C], F32, name="counts", tag="counts")
            nc.scalar.copy(counts[:], psc[:])
            # mode = first argmax of counts
            maxc = small.tile([P, 1], F32, name="maxc", tag="maxc")
            nc.vector.reduce_max(maxc[:], counts[:], axis=AX.X)
            lt01 = small.tile([P, C], F32, name="lt01", tag="lt01")
            nc.vector.tensor_scalar(
                out=lt01[:], in0=counts[:], scalar1=maxc[:], scalar2=None,
                op0=ALU.is_lt,
            )
            cand = small.tile([P, C], F32, name="cand", tag="cand")
            nc.vector.scalar_tensor_tensor(
                out=cand[:], in0=lt01[:], scalar=1e9, in1=iot[:],
                op0=ALU.mult, op1=ALU.add,
            )
            ym = small.tile([P, 1], F32, name="ym", tag="ym")
            nc.vector.tensor_reduce(ym[:], cand[:], axis=AX.X, op=ALU.min)
            nc.sync.dma_start(ym_ap[b], ym[:])

        R2 = R // 2

        def b_max(b, S):
            """Top-8 values (two half scans + merge) and their indices."""
            m16 = small.tile([P, 16], F32, name="m16", tag="m16", bufs=2)
            nc.vector.max(out=m16[:, 0:8], in_=S[:, 0:R2])
            nc.vector.max(out=m16[:, 8:16], in_=S[:, R2:R])
            m8 = small.tile([P, 8], F32, name="m8", tag="m8", bufs=2)
            nc.vector.max(out=m8[:], in_=m16[:])
            idx16 = small.tile([P, 8], U16, name="idx16", tag="idx16", bufs=2)
            nc.vector.max_index(idx16[:], m8[:], S[:])
            nc.sync.dma_start(idx_ap[b], idx16[:])

        # A blocks first (their DVE work covers the xcT DMA tail), then B
        # blocks; counts for A block b are emitted after block b+1's
        # threshold so PE counts work overlaps the DVE threshold tail.
        pending = None
        for b in range(QAB):
            S = scores(b, qTa, QA, RCHA)
            halves = a_thresh(b, S)
            if pending is not None:
                a_counts(*pending)
            pending = (b, halves)
        Sb0 = scores(0, qTb, QB, RCHB)
        a_counts(*pending)
        for b in range(QBB):
            Sn = scores(b + 1, qTb, QB, RCHB) if b + 1 < QBB else None
            b_max(b, Sb0)
            Sb0 = Sn
    nc.compile()
    return nc


def build_gm(Q, D, C, n_cores=NCORES):
    """GM launch: per-row gaussian-mixture loss against per-class means.

    Small enough to be latency-bound, so the post-exp arithmetic is batched
    across all QBB query blocks as wide [P, QBB*C] DVE ops; only the ops
    that need a per-(block, partition) scalar (exp bias, onehot, normalize,
    square+reduce) stay per-block.
    """
    DCH, QBB = D // P, Q // P
    nc = bacc.Bacc(
        "TRN2", target_bir_lowering=False, debug=False, num_devices=n_cores
    )
    qT_ap = nc.dram_tensor("qT", [P, DCH * Q], BF16, kind="ExternalInput").ap()
    muT_ap = nc.dram_tensor("muT", [P, DCH * C], BF16, kind="ExternalInput").ap()
    # emu replicated across blocks: [P, QBB*C]
    emu_ap = nc.dram_tensor("emu", [P, QBB * C], F32, kind="ExternalInput").ap()
    # qaux col b = own labels of block b; col QBB+b = -aa/2 (exp bias)
    qaux_ap = nc.dram_tensor("qaux", [P, 2 * QBB], F32, kind="ExternalInput").ap()
    io_ap = nc.dram_tensor("iotaf", [P, C], F32, kind="ExternalInput").ap()
    lg_ap = nc.dram_tensor("lgm", [P, QBB], F32, kind="ExternalOutput").ap()

    with tile.TileContext(nc) as tc, ExitStack() as ctx:
        consts = ctx.enter_context(tc.tile_pool(name="consts", bufs=1))
        small = ctx.enter_context(tc.tile_pool(name="small", bufs=1))
        psG_p = ctx.enter_context(tc.tile_pool(name="psG", bufs=2, space="PSUM"))

        tchV = consts.tile([1, 1], F32, name="tchV", tag="tchV")
        tchA = consts.tile([1, 1], F32, name="tchA", tag="tchA")
        qTt = consts.tile([P, DCH * Q], BF16, name="qTt", tag="qTt")
        nc.sync.dma_start(qTt[:], qT_ap[:])
        muTt = consts.tile([P, DCH * C], BF16, name="muTt", tag="muTt")
        nc.sync.dma_start(muTt[:], muT_ap[:])
        qauxt = consts.tile([P, 2 * QBB], F32, name="qauxt", tag="qauxt")
        nc.sync.dma_start(qauxt[:], qaux_ap[:])
        iot = consts.tile([P, C], F32, name="iot", tag="iot")
        nc.sync.dma_start(iot[:], io_ap[:])
        emut = consts.tile([P, QBB, C], F32, name="emut", tag="emut")
        nc.sync.dma_start(emut[:], emu_ap[:])
        nc.vector.tensor_copy(tchV[:], qauxt[0:1, 0:1])
        nc.vector.tensor_copy(tchV[:], iot[0:1, 0:1])
        nc.vector.tensor_copy(tchV[:], emut[0:1, 0:1, 0:1])
        nc.scalar.copy(tchA[:], qauxt[0:1, 0:1])

        eg_all = small.tile([P, QBB, C], F32, name="eg_all", tag="eg_all")
        yh_all = small.tile([P, QBB, C], F32, name="yh_all", tag="yh_all")
        # onehots only need qaux+iota: run during the qT/muT DMA fill
        for b in range(QBB):
            nc.vector.tensor_scalar(
                out=yh_all[:, b, :], in0=iot[:], scalar1=qauxt[:, b:b + 1],
                scalar2=None, op0=ALU.is_equal,
            )
        for b in range(QBB):
            psg = psG_p.tile([P, C], F32, name="psG", tag="psG")
            for d in range(DCH):
                nc.tensor.matmul(
                    psg[:],
                    qTt[:, d * Q + b * P: d * Q + (b + 1) * P],
                    muTt[:, d * C:(d + 1) * C],
                    start=(d == 0),
                    stop=(d == DCH - 1),
                )
            nc.scalar.activation(
                eg_all[:, b, :], psg[:], mybir.ActivationFunctionType.Exp,
                bias=qauxt[:, QBB + b:QBB + b + 1], scale=1.0,
            )
        piu_all = small.tile([P, QBB, C], F32, name="piu_all", tag="piu_all")
        nc.vector.tensor_mul(piu_all[:], eg_all[:], emut[:])
        srow8 = small.tile([P, QBB], F32, name="srow8", tag="srow8")
        nc.vector.reduce_sum(srow8[:], piu_all[:], axis=AX.X)
        nc.vector.tensor_scalar_add(srow8[:], srow8[:], 1e-15)
        rec8 = small.tile([P, QBB], F32, name="rec8", tag="rec8")
        nc.vector.reciprocal(rec8[:], srow8[:])
        lg8 = small.tile([P, QBB], F32, name="lg8", tag="lg8")
        for b in range(QBB):
            diff = small.tile([P, C], F32, name="diff", tag="diff", bufs=2)
            nc.vector.scalar_tensor_tensor(
                out=diff[:], in0=piu_all[:, b, :], scalar=rec8[:, b:b + 1],
                in1=yh_all[:, b, :], op0=ALU.mult, op1=ALU.subtract,
            )
            sqj = small.tile([P, C], F32, name="sqj", tag="sqj", bufs=2)
            nc.vector.tensor_mul(sqj[:], diff[:], diff[:])
            nc.vector.reduce_sum(lg8[:, b:b + 1], sqj[:], axis=AX.X)
        nc.sync.dma_start(lg_ap[:], lg8[:])
    nc.compile()
    return nc


# ---------------- host-side packing helpers ----------------

def pack_T(m):
    """[R, D] fp32 -> bf16 [P, (D//P)*R]: column block d holds rows d*P..(d+1)*P
    of m.T (i.e. element (p, d*R + r) = m[r, d*P + p])."""
    R, D = m.shape
    DCH = D // P
    mt = np.ascontiguousarray(m.T.astype(BF16_NP))  # [D, R]
    return np.ascontiguousarray(
        mt.reshape(DCH, P, R).transpose(1, 0, 2).reshape(P, DCH * R)
    )


def pack_bbhl(bb):
    """[R] fp32 -> [2, R] bf16 hi/lo split of -bb/2 (exact to ~2^-17 rel)."""
    t = (-0.5 * bb).astype(np.float32)
    hi = t.astype(BF16_NP)
    lo = (t - hi.astype(np.float32)).astype(BF16_NP)
    return np.ascontiguousarray(np.stack([hi, lo]))


def pack_cols(v):
    """[Q] -> [P, Q//P] fp32: column b = v[b*P:(b+1)*P]."""
    QB = v.shape[0] // P
    return np.ascontiguousarray(v.reshape(QB, P).T.astype(np.float32))


def mode_rows_host(vals):
    """[M, K] labels -> [M] torch.mode semantics (most frequent, smallest on
    ties)."""
    eq = vals[:, :, None] == vals[:, None, :]
    counts = eq.sum(axis=2)
    maxc = counts.max(axis=1, keepdims=True)
    masked = np.where(counts == maxc, vals, np.inf)
    return masked.min(axis=1)


_PROGRAMS = {}
LAST_EXEC_NS = None
_EXEC_NS = {}


def _get_program(key, builder):
    if key not in _PROGRAMS:
        _PROGRAMS[key] = builder()
    return _PROGRAMS[key]


def _run(nc, in_maps, phase):
    import os

    kwargs = {}
    if os.environ.get("KERNEL_TRACE"):
        kwargs = dict(trace=True, trace_cores=[0])
    t0 = _time.time()
    res = run_bass_kernel_spmd(
        nc, in_maps, core_ids=list(range(NCORES)), **kwargs
    )
    if os.environ.get("KERNEL_TIME"):
        print(f"phase {phase} dispatch+exec: {_time.time() - t0:.3f}s")
    if res.exec_time_ns:
        _EXEC_NS[phase] = res.exec_time_ns
        if res.instructions_and_trace:
            print(f"phase {phase}: {res.exec_time_ns} ns, "
                  f"trace: {res.instructions_and_trace[1]}")
    global LAST_EXEC_NS
    if len(_EXEC_NS) == 2:
        LAST_EXEC_NS = sum(_EXEC_NS.values())
    return res


def kernel(x, y, lam, perm):
    x = np.asarray(x, dtype=np.float32)
    y = np.asarray(y, dtype=np.float32)
    lam = np.float32(np.asarray(lam))
    perm = np.asarray(perm, dtype=np.int32)
    N, D = x.shape
    C = CLASSES
    x_ul = (x * lam + x[perm] * (np.float32(1.0) - lam)).astype(np.float32)
    xc = np.concatenate([x, x_ul], axis=0)
    num = xc.shape[0]

    iota_in = np.ascontiguousarray(
        np.broadcast_to(np.arange(C, dtype=np.float32), (P, C))
    )

    # ---------------- launch K: both kNN problems ----------------
    QA = N // NCORES
    QB_ = num // NCORES
    ncK = _get_program(
        ("K", num, N, QA, QB_, D),
        lambda: build_knn(num, N, QA, QB_, D, C, 11),
    )
    aa = (xc.astype(np.float64) ** 2).sum(1).astype(np.float32)
    xcT_in = pack_T(xc).reshape(P, D // P, num)
    bb_in = pack_bbhl(aa)
    ylab_in = pack_cols(y)
    in_maps = []
    for c in range(NCORES):
        in_maps.append(
            {
                "xcT": xcT_in,
                "qTa": pack_T(x_ul[c * QA:(c + 1) * QA]),
                "qTb": pack_T(xc[c * QB_:(c + 1) * QB_]),
                "bbhl": bb_in,
                "ylab": ylab_in,
                "iotaf": iota_in,
            }
        )
    resK = _run(ncK, in_maps, "K")
    y_ul = np.concatenate(
        [r["ymode"].reshape(QA) for r in resK.results]
    ).astype(np.float32)
    # idxo[b, p, j] = j-th nearest ref of query (b*128 + p); rank 0 is self.
    idx_all = np.concatenate(
        [r["idxo"].reshape(QB_, 8) for r in resK.results]
    ).astype(np.int64)

    # ---------------- host glue: per-class means, 3-NN mode ----------------
    yc = np.concatenate([y, y_ul], axis=0)
    yi = yc.astype(np.int32)
    counts = np.bincount(yi, minlength=C).astype(np.float32)
    mu = np.zeros((C, D), dtype=np.float32)
    np.add.at(mu, yi, xc)
    mu = mu / np.maximum(counts, 1.0)[:, None]
    bbm = (mu.astype(np.float64) ** 2).sum(1)
    emu = (np.exp(-bbm / 2.0) * (counts > 0)).astype(np.float32)
    QBB = (num // NCORES) // P
    emu_in = np.ascontiguousarray(
        np.broadcast_to(np.tile(emu, QBB), (P, QBB * C))
    )
    muT_in = pack_T(mu)
    y_ng = mode_rows_host(yc[idx_all[:, 1:4]]).astype(np.float32)

    # ---------------- launch G: gm loss rows ----------------
    ncG = _get_program(("G", QB_, D), lambda: build_gm(QB_, D, C))
    in_maps = []
    for c in range(NCORES):
        sl = slice(c * QB_, (c + 1) * QB_)
        qaux = np.concatenate(
            [pack_cols(yc[sl]), pack_cols(-0.5 * aa[sl])], axis=1
        ).astype(np.float32)
        in_maps.append(
            {
                "qT": pack_T(xc[c * QB_:(c + 1) * QB_]),
                "muT": muT_in,
                "emu": emu_in,
                "qaux": np.ascontiguousarray(qaux),
                "iotaf": iota_in,
            }
        )
    resG = _run(ncG, in_maps, "G")
    # lgm[p, b] = per-row loss of query (b*128 + p) on that core
    lgm_rows = np.concatenate(
        [r["lgm"].reshape(P, QB_ // P).T.reshape(QB_) for r in resG.results]
    )

    loss_gm = np.float32(lgm_rows.mean(dtype=np.float64))
    loss_knn = np.float32(((y_ng - yc) ** 2).mean(dtype=np.float64))
    return np.float32(loss_gm + np.float32(0.01) * loss_knn)
